# revision 1
# baseline (speedup 1.0000x reference)
"""NT-Xent (SimCLR) contrastive loss on 8 Trainium2 NeuronCores — v2.

Symmetric half-band design. exp(sim) is symmetric, so each global row i only
computes columns at circular distance d = j-i mod 2N in [1, 4096]: every
unordered pair lands on exactly one core, except d == 4096 (the positive
pair) which lands on both and is corrected on the host. Per core this halves
both the PE matmul work and the ACT exp work vs the full-matrix version.

Core c owns rows [c*1024, (c+1)*1024). Its input is z^T column-rotated by
c*1024 and truncated to the first 5120 columns (the union of all its bands),
so the program is identical on every core. Row-tile rt (128 rows) computes
the band of 33 column-tiles [rt*128, rt*128+4224): 32 full tiles in 4 chunks
of 1024 plus a 128-wide "wedge" (distance ~4096) tile; the first tile masks
d <= 0 and the wedge masks d > 4096 (additive -1e5 pre-exp => exp -> 0).

Row sums of exp come free via the activation's accum_out. Column sums (the
transposed half of each pair's contribution) are ones-vector matmuls over
the exp'd chunk, accumulated at PSUM partition rt and DMA'd out per chunk.
The host assembles the 2N denominators from row/col partial sums, subtracts
the double-counted positive exp, and takes mean(log(D) - 2*pos).
"""

import sys

for _p in ("/opt/trn_rl_repo",):
    if _p not in sys.path:
        sys.path.insert(0, _p)

import numpy as np

import concourse.bass as bass
import concourse.tile as tile
from concourse import bacc, mybir
from concourse.bass_utils import run_bass_kernel_spmd

F32 = mybir.dt.float32
AF = mybir.ActivationFunctionType

N_CORES = 8
N = 4096
D = 256
TWO_N = 2 * N            # 8192 rows of sim
ROWS = TWO_N // N_CORES  # 1024 rows per core
NBLK = 5                 # 5 column blocks of 1024 -> cols [0, 5120)
COLS = NBLK * 1024       # 5120 columns staged per core
BAND = 4224              # 33 tiles: band columns per row-tile
NEG_MASK = -1.0e5        # additive pre-exp mask; exp underflows to 0
MMDT = mybir.dt.float32r

_CACHE = {}
LAST_RESULTS = None


def _build_nc() -> bass.Bass:
    nc = bacc.Bacc("TRN2", num_devices=N_CORES)

    zt_d = nc.dram_tensor("zt", [D, COLS], F32, kind="ExternalInput")
    mlo_d = nc.dram_tensor("mlo", [128, 128], F32, kind="ExternalInput")
    mup_d = nc.dram_tensor("mup", [128, 128], F32, kind="ExternalInput")
    rows_d = nc.dram_tensor("rows", [128, 8], F32, kind="ExternalOutput")
    cols_d = nc.dram_tensor("cols", [8, BAND], F32, kind="ExternalOutput")
    pos_d = nc.dram_tensor("pos", [1, ROWS], F32, kind="ExternalOutput")
    u_d = nc.dram_tensor("uscratch", [1, COLS], F32)  # internal DRAM bounce

    with tile.TileContext(nc) as tc:
        with (
            tc.tile_pool(name="big", bufs=1) as big,
            tc.tile_pool(name="wraw", bufs=3) as wraw,
            tc.tile_pool(name="wsq", bufs=2) as wsq,
            tc.tile_pool(name="wsm", bufs=2) as wsm,
            tc.tile_pool(name="we", bufs=3) as we,
            tc.tile_pool(name="small", bufs=1) as small,
            tc.tile_pool(name="stat", bufs=1) as stat,
            tc.tile_pool(name="ps", bufs=3, space="PSUM") as ps,
            tc.tile_pool(name="aux", bufs=1, space="PSUM") as aux,
        ):
            # normalized z^T (f32r) for all matmuls; persistent
            zt0 = big.tile([128, COLS], MMDT, tag="zt0")  # dims 0:128
            zt1 = big.tile([128, COLS], MMDT, tag="zt1")  # dims 128:256
            mask_lo = small.tile([128, 128], F32, tag="mlo")
            nc.sync.dma_start(out=mask_lo[:, :], in_=mlo_d.ap()[:, :])
            mask_up = small.tile([128, 128], F32, tag="mup")
            nc.sync.dma_start(out=mask_up[:, :], in_=mup_d.ap()[:, :])

            ones_f = small.tile([128, 1], F32, tag="onesf")
            nc.vector.memset(ones_f[:, :], 1.0)
            ones = small.tile([128, 1], MMDT, tag="ones")
            nc.vector.tensor_copy(ones[:, :], ones_f[:, :])
            # K=1 stationary row of ones for the broadcast matmul
            onesr_f = small.tile([1, 128], F32, tag="onesrf")
            nc.vector.memset(onesr_f[:, :], 1.0)
            onesr = small.tile([1, 128], MMDT, tag="onesr")
            nc.vector.tensor_copy(onesr[:, :], onesr_f[:, :])
            # selector stationaries: ones_sel[rt] = [128, 8], col rt ones.
            # matmul with lhsT=ones_sel[rt] lands E_rt's colsum at psum
            # partition rt (PE out base partition must be 0/32/64, so the 8
            # row-tiles accumulate into one [8, w] tile instead).
            self_f = small.tile([128, 8], F32, tag="self")
            ones_sel = []
            for rt in range(8):
                nc.vector.memset(self_f[:, :], 0.0)
                nc.vector.memset(self_f[:, rt : rt + 1], 1.0)
                s = small.tile([128, 8], MMDT, tag=f"sel{rt}")
                nc.vector.tensor_copy(s[:, :], self_f[:, :])
                ones_sel.append(s)

            # per-(row-tile, chunk) partial row sums of exp: col rt*5+c
            rowsums = stat.tile([128, 40], F32, tag="rsum")

            def load(b):
                sl = slice(b * 1024, (b + 1) * 1024)
                r0 = wraw.tile([128, 1024], F32, tag="r0")
                r1 = wraw.tile([128, 1024], F32, tag="r1")
                nc.sync.dma_start(out=r0[:, :], in_=zt_d.ap()[0:128, sl])
                nc.sync.dma_start(out=r1[:, :], in_=zt_d.ap()[128:256, sl])
                return r0, r1

            def prologue(b, raw):
                """Normalize columns of block b: zt{0,1}[:, blk] =
                raw / ||col||. Squares on gpsimd (idle engine); column sums
                via ones-matmul; 1/sqrt via ln+exp on ACT in a [16, 64]
                transposed layout; broadcast back via a K=1 ones matmul."""
                sl = slice(b * 1024, (b + 1) * 1024)
                r0, r1 = raw
                # squares split DVE/Pool so the two run concurrently (Pool's
                # tensor ops are ~2x slower per element than DVE's)
                sq0 = wsq.tile([128, 1024], F32, tag="sq0")
                sq1 = wsq.tile([128, 1024], F32, tag="sq1")
                sqs = wsq.tile([128, 1024], MMDT, tag="sqs")
                nc.vector.tensor_mul(sq0[:, :], r0[:, :], r0[:, :])
                nc.gpsimd.tensor_mul(sq1[:, :], r1[:, :], r1[:, :])
                nc.vector.tensor_add(sqs[:, :], sq0[:, :], sq1[:, :])

                # norm accumulator borrows a main-pool tile (row 0) so the
                # aux pool stays dedicated to colsums — sharing one aux
                # buffer serialized every prologue behind the previous
                # chunk's colsum drain.
                nrm = ps.tile([128, 1024], F32, tag="mm")
                for bb in range(2):
                    bs = slice(bb * 512, (bb + 1) * 512)
                    nc.tensor.matmul(
                        nrm[0:1, bs], ones[:, :], sqs[:, bs],
                        start=True, stop=True,
                    )
                # evacuate [1,1024] psum -> sbuf, then DMA-transpose into
                # 16 partitions of sstb
                ssqr = wsm.tile([1, 1024], F32, tag="ssqr")
                nc.vector.tensor_copy(ssqr[0:1, :], nrm[0:1, :])
                sstb = wsm.tile([16, 64], F32, tag="sstb")
                nc.sync.dma_start(out=sstb[0:16, :], in_=ssqr[0:1, :])
                # u = exp(-0.5*ln(ssq)) = 1/sqrt(ssq)
                lnt = wsm.tile([16, 64], F32, tag="lnt")
                ut = wsm.tile([16, 64], F32, tag="ut")
                nc.scalar.activation(lnt[:, :], sstb[:, :], AF.Ln)
                nc.scalar.activation(ut[:, :], lnt[:, :], AF.Exp, scale=-0.5)
                # scatter u to DRAM in column order, then broadcast-read
                # across all 128 partitions (DMA-only; keeps the broadcast
                # off the in-order PE queue so it can't block the next
                # chunk's matmuls)
                u_out = bass.AP(
                    tensor=u_d.ap().tensor,
                    offset=b * 1024,
                    ap=[[64, 16], [1, 64]],
                )
                nc.sync.dma_start(out=u_out, in_=ut[:, :])
                ubc = wsm.tile([128, 1024], F32, tag="ubc")
                u_sl = u_d.ap()[0:1, sl]
                u_bcast = bass.AP(
                    tensor=u_sl.tensor,
                    offset=u_sl.offset,
                    ap=[[0, 128]] + list(u_sl.ap[1:]),
                )
                nc.sync.dma_start(out=ubc[:, :], in_=u_bcast)
                nc.vector.tensor_mul(zt0[:, sl], r0[:, :], ubc[:, :])
                nc.vector.tensor_mul(zt1[:, sl], r1[:, :], ubc[:, :])

            def cpass(c):
                """Chunk c (band-relative cols [c*1024, (c+1)*1024)) for all
                8 row-tiles: matmul -> (mask) -> exp+rowsum -> colsum."""
                csum = aux.tile([8, 1024], F32, tag="cs")
                etiles = {}

                def colsum(rt):
                    e = etiles.pop(rt)
                    for bb in range(2):
                        bs = slice(bb * 512, (bb + 1) * 512)
                        nc.tensor.matmul(
                            csum[0:8, bs], ones_sel[rt][:, :], e[:, bs],
                            start=(rt == 0), stop=(rt == 7),
                        )

                for rt in range(8):
                    o = rt * 128 + c * 1024
                    pq = ps.tile([128, 1024], F32, tag="mm")
                    for ki, zk in enumerate((zt0, zt1)):
                        lhsT = zk[:, rt * 128 : rt * 128 + 128]
                        for bb in range(2):
                            nc.tensor.matmul(
                                pq[:, bb * 512 : (bb + 1) * 512],
                                lhsT,
                                zk[:, o + bb * 512 : o + (bb + 1) * 512],
                                start=(ki == 0),
                                stop=(ki == 1),
                            )
                    if c == 0:
                        # first band tile: mask distance <= 0 (diag+lower)
                        nc.vector.tensor_add(
                            pq[:, 0:128], pq[:, 0:128], mask_lo[:, :]
                        )
                    e = we.tile([128, 1024], MMDT, tag="e")
                    etiles[rt] = e
                    nc.scalar.activation(
                        e[:, :], pq[:, :], AF.Exp, scale=2.0,
                        accum_out=rowsums[:, rt * 5 + c : rt * 5 + c + 1],
                    )
                    # colsum two row-tiles back so exp has drained
                    if rt >= 2:
                        colsum(rt - 2)
                colsum(6)
                colsum(7)
                csb = wsm.tile([8, 1024], F32, tag="csb")
                nc.vector.tensor_copy(csb[:, :], csum[:, :])
                nc.sync.dma_start(
                    out=cols_d.ap()[0:8, c * 1024 : (c + 1) * 1024],
                    in_=csb[:, :],
                )

            def wedge():
                """Distance-4096 tile per row-tile: cols rt*128+[4096,4224),
                keep t <= p (mask strict upper)."""
                csum = aux.tile([8, 1024], F32, tag="cs")
                for rt in range(8):
                    o = rt * 128 + 4096
                    pq = ps.tile([128, 1024], F32, tag="mm")
                    for ki, zk in enumerate((zt0, zt1)):
                        nc.tensor.matmul(
                            pq[:, 0:128],
                            zk[:, rt * 128 : rt * 128 + 128],
                            zk[:, o : o + 128],
                            start=(ki == 0),
                            stop=(ki == 1),
                        )
                    nc.vector.tensor_add(
                        pq[:, 0:128], pq[:, 0:128], mask_up[:, :]
                    )
                    e = we.tile([128, 1024], MMDT, tag="e")
                    nc.scalar.activation(
                        e[:, 0:128], pq[:, 0:128], AF.Exp, scale=2.0,
                        accum_out=rowsums[:, rt * 5 + 4 : rt * 5 + 5],
                    )
                    nc.tensor.matmul(
                        csum[0:8, 0:128], ones_sel[rt][:, :], e[:, 0:128],
                        start=(rt == 0), stop=(rt == 7),
                    )
                csb = wsm.tile([8, 1024], F32, tag="csb")
                nc.vector.tensor_copy(csb[:, 0:128], csum[:, 0:128])
                nc.sync.dma_start(
                    out=cols_d.ap()[0:8, 4096:4224], in_=csb[:, 0:128]
                )

            def pos_pass():
                """posdot[i] = z_i . z_{i+4096} for local rows 0..1023."""
                posps = aux.tile([8, 1024], F32, tag="cs")
                for ki, zk in enumerate((zt0, zt1)):
                    prod = wsq.tile([128, 1024], MMDT, tag="sq0")
                    nc.vector.tensor_mul(
                        prod[:, :], zk[:, 0:ROWS], zk[:, 4096 : 4096 + ROWS]
                    )
                    for bb in range(2):
                        bs = slice(bb * 512, (bb + 1) * 512)
                        nc.tensor.matmul(
                            posps[0:1, bs], ones[:, :], prod[:, bs],
                            start=(ki == 0), stop=(ki == 1),
                        )
                psb = wsm.tile([1, 1024], F32, tag="psb")
                nc.vector.tensor_copy(psb[0:1, :], posps[0:1, :])
                nc.sync.dma_start(out=pos_d.ap()[:, :], in_=psb[0:1, :])

            # staggered emission: chunk c needs normalized blocks c, c+1.
            # prologues run one full cpass ahead so their multi-engine chain
            # latency hides inside the previous chunk's compute.
            raw0 = load(0)
            raw1 = load(1)
            prologue(0, raw0)
            prologue(1, raw1)
            raw2 = load(2)
            prologue(2, raw2)
            cpass(0)
            raw3 = load(3)
            prologue(3, raw3)
            cpass(1)
            raw4 = load(4)
            prologue(4, raw4)
            cpass(2)
            pos_pass()
            cpass(3)
            wedge()

            # reduce rowsums [128, 8x5] -> [128, 8] and ship
            rsum8 = stat.tile([128, 8], F32, tag="rs8")
            nc.vector.tensor_reduce(
                rsum8[:, :],
                rowsums[:, :].rearrange("p (a b) -> p a b", b=5),
                axis=mybir.AxisListType.X,
                op=mybir.AluOpType.add,
            )
            nc.sync.dma_start(out=rows_d.ap()[:, :], in_=rsum8[:, :])

    _patch_act_table_loads(nc)
    nc.compile()
    return nc


def _act_set_id_with_exp_and_ln(nc) -> int:
    from concourse.hw_specs import get_activation_tables

    tabs = get_activation_tables(nc.m.arch)
    for i, (name, fns) in enumerate(tabs.items()):
        if AF.Exp in fns and AF.Ln in fns:
            return i
    raise RuntimeError("no activation table set with both Exp and Ln")


def _patch_act_table_loads(nc) -> None:
    # Load the combined exp+ln table once instead of per-switch reloads.
    combined_set_id = _act_set_id_with_exp_and_ln(nc)

    def _single_act_table_load():
        for blk in nc.main_func.blocks:
            insts = list(blk.instructions)
            for i, ins in enumerate(insts):
                if isinstance(ins, mybir.InstActivation):
                    load = mybir.InstLoadActFuncSet(
                        name=nc.get_next_instruction_name(),
                        act_func_set_id=combined_set_id,
                        ins=[],
                        outs=[],
                    )
                    load.engine = mybir.EngineType.Activation
                    insts.insert(i, load)
                    blk.instructions = insts
                    break

    nc.insert_act_table_loads = _single_act_table_load


def _get_nc() -> bass.Bass:
    if "nc" not in _CACHE:
        _CACHE["nc"] = _build_nc()
    return _CACHE["nc"]


def _masks():
    p = np.arange(128)[:, None]
    t = np.arange(128)[None, :]
    mlo = np.where(t <= p, NEG_MASK, 0.0).astype(np.float32)  # keep d >= 1
    mup = np.where(t > p, NEG_MASK, 0.0).astype(np.float32)  # keep d <= 4096
    return mlo, mup


def kernel(emb_i: np.ndarray, emb_j: np.ndarray) -> np.ndarray:
    global LAST_RESULTS
    z = np.concatenate(
        [np.asarray(emb_i, dtype=np.float32), np.asarray(emb_j, dtype=np.float32)],
        axis=0,
    )  # [8192, 256]
    zt = np.ascontiguousarray(z.T)  # [256, 8192]
    mlo, mup = _masks()

    in_maps = []
    for c in range(N_CORES):
        ztc = zt if c == 0 else np.roll(zt, -c * ROWS, axis=1)
        ztc = np.ascontiguousarray(ztc[:, :COLS])
        in_maps.append({"zt": ztc, "mlo": mlo, "mup": mup})

    nc = _get_nc()
    LAST_RESULTS = run_bass_kernel_spmd(nc, in_maps, list(range(N_CORES)))

    den = np.zeros(TWO_N, dtype=np.float64)
    posd = np.zeros(TWO_N, dtype=np.float64)
    band_j = np.arange(BAND)
    for c in range(N_CORES):
        r = LAST_RESULTS.results[c]
        rows = np.asarray(r["rows"], dtype=np.float64)  # [128, 8] (p, rt)
        cols = np.asarray(r["cols"], dtype=np.float64)  # [8, BAND]
        pos = np.asarray(r["pos"], dtype=np.float64)[0]  # [1024]
        den[c * ROWS : (c + 1) * ROWS] += rows.T.reshape(-1)
        for rt in range(8):
            g = (c * ROWS + rt * 128 + band_j) % TWO_N
            den[g] += cols[rt]
        posd[c * ROWS : (c + 1) * ROWS] = pos
    # distance-4096 pairs were computed by both endpoints: subtract once
    den -= np.exp(2.0 * posd)
    loss = np.mean(np.log(den) - 2.0 * posd)
    return np.array(loss, dtype=np.float32)



# revision 18
# speedup vs baseline: 1.6533x; 1.6533x over previous
"""NT-Xent (SimCLR) contrastive loss on 8 Trainium2 NeuronCores — v3 (fp8).

Symmetric half-band design as v2: exp(sim) is symmetric, so each global row i
only computes columns at circular distance d = j-i mod 2N in [1, 4096]; every
unordered pair lands on exactly one core except d == 4096 (the positive pair),
which lands on both and is corrected on the host.

v3 changes vs v2:
- z is normalized on the HOST (f32) and shipped as fp8e4m3 in a DoubleRow
  plane layout [128, 2, COLS] (partition p holds dims p and p+128). This
  deletes the whole on-device normalization pipeline (squares, norm matmuls,
  rsqrt chain, broadcast DMAs) that caused multi-us dependency bubbles, and
  cuts input DMA 4x.
- All matmuls run fp8 DoubleRow (0.5 cyc/row, K=256 in one pass): the sim
  matmul needs one instruction per 512 output cols, and the exp'd band tiles
  of two adjacent row-tiles are column-summed in one paired matmul.
- exp runs on ACT from [128,1536] PSUM tiles (3 per row-tile: 1536+1536+1152)
  with accum_out giving f32 row sums for free; e is written back as fp8 only
  for the colsum matmul. ACT is the bottleneck engine (~39 us busy/core).
- Column sums go per-pair straight from PSUM to DRAM (rows 2p,2p+1 of the
  [8, BAND] output), no DVE evacuation.

Host assembles den from f32 row sums + fp8-rounded col sums, subtracts the
double-counted positive exp, and takes mean(log(D) - 2*pos) with pos in f32.
"""

import sys

for _p in ("/opt/trn_rl_repo",):
    if _p not in sys.path:
        sys.path.insert(0, _p)

import ml_dtypes
import numpy as np

import concourse.bass as bass
import concourse.tile as tile
from concourse import bacc, mybir
from concourse.bass_utils import run_bass_kernel_spmd

F32 = mybir.dt.float32
F8 = mybir.dt.float8e4
AF = mybir.ActivationFunctionType
DR = mybir.MatmulPerfMode.DoubleRow
NP_F8 = ml_dtypes.float8_e4m3

N_CORES = 8
N = 4096
D = 256
TWO_N = 2 * N            # 8192 rows of sim
ROWS = TWO_N // N_CORES  # 1024 rows per core
COLS = 5120              # rotated columns staged per core
BAND = 4224              # band columns per 128-row tile (4096 + 128 wedge)
NEG_MASK = -128.0        # fp8-exact; exp(2*(sim-128)) underflows to 0
SEGS = ((0, 1536), (1536, 1536), (3072, 1152))  # band segments per row-tile

_CACHE = {}
LAST_RESULTS = None


def _plane3(base: bass.AP, off: int, plane_stride: int, w: int) -> bass.AP:
    """[128, 2, w] DoubleRow view of a plane-major [128, 2*S] sbuf tile."""
    return bass.AP(
        tensor=base.tensor,
        offset=base.offset + off,
        ap=[list(base.ap[0]), [plane_stride, 2], [1, w]],
    )


def _build_nc() -> bass.Bass:
    nc = bacc.Bacc("TRN2", num_devices=N_CORES)

    z_d = nc.dram_tensor("z8", [128, 2 * COLS], F8, kind="ExternalInput")
    # masks as PE accumulation operands: pq[p,t] += mA[t,p] via rhs=identity
    mlo_d = nc.dram_tensor("mlo", [128, 128], F8, kind="ExternalInput")
    mup_d = nc.dram_tensor("mup", [128, 128], F8, kind="ExternalInput")
    ident_d = nc.dram_tensor("ident", [128, 128], F8, kind="ExternalInput")
    rows_d = nc.dram_tensor("rows", [128, 8], F32, kind="ExternalOutput")
    cols_d = nc.dram_tensor("cols", [8, BAND], F32, kind="ExternalOutput")

    with tile.TileContext(nc) as tc:
        with (
            tc.tile_pool(name="big", bufs=1) as big,
            tc.tile_pool(name="ep", bufs=2) as ep,
            tc.tile_pool(name="cg", bufs=2) as cg,
            tc.tile_pool(name="small", bufs=1) as small,
            tc.tile_pool(name="ps", bufs=2, space="PSUM") as ps,
            tc.tile_pool(name="cs", bufs=2, space="PSUM") as cs,
        ):
            z8 = big.tile([128, 2 * COLS], F8, tag="z8")
            # 4 input DMAs (per-plane halves) so rt 0 can start early
            H = COLS // 2
            for pl in range(2):
                for h in range(2):
                    sl = slice(pl * COLS + h * H, pl * COLS + (h + 1) * H)
                    nc.sync.dma_start(out=z8[:, sl], in_=z_d.ap()[:, sl])

            mask_lo = small.tile([128, 128], F8, tag="mlo")
            nc.sync.dma_start(out=mask_lo[:, :], in_=mlo_d.ap()[:, :])
            mask_up = small.tile([128, 128], F8, tag="mup")
            nc.sync.dma_start(out=mask_up[:, :], in_=mup_d.ap()[:, :])
            ident = small.tile([128, 128], F8, tag="ident")
            nc.sync.dma_start(out=ident[:, :], in_=ident_d.ap()[:, :])

            # pair selector [128, 2, 128]: plane 0 -> row 0 (even row-tile),
            # plane 1 -> row 1 (odd row-tile); every pair's colsums land at
            # psum partitions 0:2 (engine partition-base must be 0/32/64/96).
            # Full 128-col stationary: narrower ones fail the LDW ISA check.
            self_f = small.tile([128, 256], F32, tag="selftmp")
            nc.vector.memset(self_f[:, :], 0.0)
            nc.vector.memset(self_f[:, 0:1], 1.0)
            nc.vector.memset(self_f[:, 129:130], 1.0)
            sel = small.tile([128, 256], F8, tag="sel")
            nc.vector.tensor_copy(sel[:, :], self_f[:, :])

            rowsums = small.tile([128, 24], F32, tag="rsum")
            zbase = z8[:, :]

            et = None
            for rt in range(8):
                if rt % 2 == 0:
                    et = ep.tile([128, 2 * BAND], F8, tag="e")
                pl = rt % 2
                ebase = et[:, :]
                for si, (off, w) in enumerate(SEGS):
                    pq = ps.tile([128, 1536], F32, tag="pq")
                    lhsT = _plane3(zbase, rt * 128, COLS, 128)
                    for sub0 in range(0, w, 512):
                        sw = min(512, w - sub0)
                        # the masked sub-tiles keep their accumulation group
                        # open for the mask matmul below
                        masked = (si == 0 and sub0 == 0) or (
                            si == 2 and sub0 == 1024
                        )
                        nc.tensor.matmul(
                            pq[:, sub0 : sub0 + sw],
                            lhsT,
                            _plane3(zbase, rt * 128 + off + sub0, COLS, sw),
                            start=True,
                            stop=not masked,
                            perf_mode=DR,
                        )
                    if si == 0:
                        # first band tile: mask distance <= 0 (diag+lower):
                        # pq[p,t] += NEG*1[t<=p] as a matmul (lhsT[k,m] =
                        # NEG*1[k<=m], rhs=I), keeping the mask on the PE so
                        # no other engine gates the exp
                        nc.tensor.matmul(
                            pq[:, 0:128],
                            mask_lo[:, :],
                            ident[:, :],
                            start=False,
                            stop=True,
                            skip_group_check=True,
                        )
                    if si == 2:
                        # wedge (distance ~4096): mask d > 4096
                        nc.tensor.matmul(
                            pq[:, 1024:1152],
                            mask_up[:, :],
                            ident[:, :],
                            start=False,
                            stop=True,
                            skip_group_check=True,
                        )
                    nc.scalar.activation(
                        et[:, pl * BAND + off : pl * BAND + off + w],
                        pq[:, 0:w],
                        AF.Exp,
                        scale=2.0,
                        accum_out=rowsums[:, rt * 3 + si : rt * 3 + si + 1],
                    )
                    if pl == 1:
                        # colsum this segment for the (rt-1, rt) pair as soon
                        # as both exps have been issued; DMA can't read PSUM,
                        # so bounce rows 2p:2p+2 through sbuf staging
                        p = rt // 2
                        if si == 0:
                            cstage = cg.tile([2, BAND], F32, tag="cstage")
                        for sub0 in range(off, off + w, 512):
                            sw = min(512, off + w - sub0)
                            cst = cs.tile([128, 512], F32, tag="cs")
                            nc.tensor.matmul(
                                cst[0:128, 0:sw],
                                _plane3(sel[:, :], 0, 128, 128),
                                _plane3(ebase, sub0, BAND, sw),
                                start=True,
                                stop=True,
                                perf_mode=DR,
                            )
                            nc.vector.tensor_copy(
                                cstage[0:2, sub0 : sub0 + sw],
                                cst[0:2, 0:sw],
                            )
                        if si == 2:
                            nc.sync.dma_start(
                                out=cols_d.ap()[2 * p : 2 * p + 2, :],
                                in_=cstage[0:2, :],
                            )

            # reduce rowsums [128, 8x3] -> [128, 8] and ship
            rsum8 = small.tile([128, 8], F32, tag="rs8")
            nc.vector.tensor_reduce(
                rsum8[:, :],
                rowsums[:, :].rearrange("p (a b) -> p a b", b=3),
                axis=mybir.AxisListType.X,
                op=mybir.AluOpType.add,
            )
            nc.sync.dma_start(out=rows_d.ap()[:, :], in_=rsum8[:, :])

    _patch_act_table_loads(nc)
    nc.compile()
    return nc


def _act_set_id_with_exp(nc) -> int:
    from concourse.hw_specs import get_activation_tables

    tabs = get_activation_tables(nc.m.arch)
    for i, (name, fns) in enumerate(tabs.items()):
        if AF.Exp in fns:
            return i
    raise RuntimeError("no activation table set with Exp")


def _patch_act_table_loads(nc) -> None:
    # Load the exp table once up front instead of per-switch reloads.
    set_id = _act_set_id_with_exp(nc)

    def _single_act_table_load():
        for blk in nc.main_func.blocks:
            insts = list(blk.instructions)
            for i, ins in enumerate(insts):
                if isinstance(ins, mybir.InstActivation):
                    load = mybir.InstLoadActFuncSet(
                        name=nc.get_next_instruction_name(),
                        act_func_set_id=set_id,
                        ins=[],
                        outs=[],
                    )
                    load.engine = mybir.EngineType.Activation
                    insts.insert(i, load)
                    blk.instructions = insts
                    break

    nc.insert_act_table_loads = _single_act_table_load


def _get_nc() -> bass.Bass:
    if "nc" not in _CACHE:
        _CACHE["nc"] = _build_nc()
    return _CACHE["nc"]


def _masks():
    # lhsT constants for the mask matmuls (rhs = identity):
    # pq[p, t] += mlo[t, p], so mlo[k, m] = NEG iff k <= m masks t <= p
    # (keeps d >= 1); mup[k, m] = NEG iff k > m masks t > p (keeps d <= 4096)
    k = np.arange(128)[:, None]
    m = np.arange(128)[None, :]
    mlo = np.where(k <= m, NEG_MASK, 0.0).astype(NP_F8)
    mup = np.where(k > m, NEG_MASK, 0.0).astype(NP_F8)
    ident = np.eye(128, dtype=np.float32).astype(NP_F8)
    return mlo, mup, ident


def kernel(emb_i: np.ndarray, emb_j: np.ndarray) -> np.ndarray:
    global LAST_RESULTS
    z = np.concatenate(
        [np.asarray(emb_i, dtype=np.float32), np.asarray(emb_j, dtype=np.float32)],
        axis=0,
    )  # [8192, 256]
    z /= np.maximum(np.sqrt((z * z).sum(axis=1, keepdims=True)), 1e-12)
    z8 = z.astype(NP_F8)           # device values, exact
    z8f = z8.astype(np.float32)
    zt8 = np.ascontiguousarray(z8.T)  # [256, 8192] fp8
    mlo, mup, ident = _masks()

    in_maps = []
    for c in range(N_CORES):
        ztc = zt8 if c == 0 else np.roll(zt8, -c * ROWS, axis=1)
        ztc = ztc[:, :COLS]
        # DoubleRow plane layout: [128, 2*COLS], partition p = dims (p, p+128)
        buf = np.ascontiguousarray(
            np.concatenate([ztc[:128, :], ztc[128:, :]], axis=1)
        )
        in_maps.append({"z8": buf, "mlo": mlo, "mup": mup, "ident": ident})

    nc = _get_nc()
    LAST_RESULTS = run_bass_kernel_spmd(nc, in_maps, list(range(N_CORES)))

    den = np.zeros(TWO_N, dtype=np.float64)
    band_j = np.arange(BAND)
    for c in range(N_CORES):
        r = LAST_RESULTS.results[c]
        rows = np.asarray(r["rows"], dtype=np.float64)  # [128, 8] (p, rt)
        cols = np.asarray(r["cols"], dtype=np.float64)  # [8, BAND]
        den[c * ROWS : (c + 1) * ROWS] += rows.T.reshape(-1)
        for rt in range(8):
            g = (c * ROWS + rt * 128 + band_j) % TWO_N
            den[g] += cols[rt]

    idx = np.arange(TWO_N)
    pidx = (idx + N) % TWO_N
    # distance-4096 pairs were computed by both endpoints: subtract once,
    # using the same fp8 z the device saw
    pos8 = (z8f[idx] * z8f[pidx]).sum(axis=1, dtype=np.float64)
    den -= np.exp(2.0 * pos8)
    # the loss's positive term uses full-precision z
    pos = (z[idx].astype(np.float64) * z[pidx].astype(np.float64)).sum(axis=1)
    loss = np.mean(np.log(den) - 2.0 * pos)
    return np.array(loss, dtype=np.float32)


# revision 26
# speedup vs baseline: 1.7191x; 1.0398x over previous
"""NT-Xent (SimCLR) contrastive loss on 8 Trainium2 NeuronCores — v3 (fp8).

Symmetric half-band design as v2: exp(sim) is symmetric, so each global row i
only computes columns at circular distance d = j-i mod 2N in [1, 4096]; every
unordered pair lands on exactly one core except d == 4096 (the positive pair),
which lands on both and is corrected on the host.

v3 changes vs v2:
- z is normalized on the HOST (f32) and shipped as fp8e4m3 in a DoubleRow
  plane layout [128, 2, COLS] (partition p holds dims p and p+128). This
  deletes the whole on-device normalization pipeline (squares, norm matmuls,
  rsqrt chain, broadcast DMAs) that caused multi-us dependency bubbles, and
  cuts input DMA 4x.
- All matmuls run fp8 DoubleRow (0.5 cyc/row, K=256 in one pass): the sim
  matmul needs one instruction per 512 output cols, and the exp'd band tiles
  of two adjacent row-tiles are column-summed in one paired matmul.
- exp runs on ACT from [128,1536] PSUM tiles (3 per row-tile: 1536+1536+1152)
  with accum_out giving f32 row sums for free; e is written back as fp8 only
  for the colsum matmul. ACT is the bottleneck engine (~39 us busy/core).
- Column sums go per-pair straight from PSUM to DRAM (rows 2p,2p+1 of the
  [8, BAND] output), no DVE evacuation.

Host assembles den from f32 row sums + fp8-rounded col sums, subtracts the
double-counted positive exp, and takes mean(log(D) - 2*pos) with pos in f32.
"""

import sys

for _p in ("/opt/trn_rl_repo",):
    if _p not in sys.path:
        sys.path.insert(0, _p)

import ml_dtypes
import numpy as np

import concourse.bass as bass
import concourse.tile as tile
from concourse import bacc, mybir
from concourse.bass_utils import run_bass_kernel_spmd

F32 = mybir.dt.float32
F8 = mybir.dt.float8e4
AF = mybir.ActivationFunctionType
DR = mybir.MatmulPerfMode.DoubleRow
NP_F8 = ml_dtypes.float8_e4m3

N_CORES = 8
N = 4096
D = 256
TWO_N = 2 * N            # 8192 rows of sim
ROWS = TWO_N // N_CORES  # 1024 rows per core
COLS = 5120              # rotated columns staged per core
BAND = 4224              # band columns per 128-row tile (4096 + 128 wedge)
NEG_MASK = -128.0        # fp8-exact; exp(2*(sim-128)) underflows to 0
SEGS = ((0, 1536), (1536, 1536), (3072, 1152))  # band segments per row-tile

_CACHE = {}
LAST_RESULTS = None


def _plane3(base: bass.AP, off: int, plane_stride: int, w: int) -> bass.AP:
    """[128, 2, w] DoubleRow view of a plane-major [128, 2*S] sbuf tile."""
    return bass.AP(
        tensor=base.tensor,
        offset=base.offset + off,
        ap=[list(base.ap[0]), [plane_stride, 2], [1, w]],
    )


def _build_nc() -> bass.Bass:
    nc = bacc.Bacc("TRN2", num_devices=N_CORES)

    z_d = nc.dram_tensor("z8", [128, 2 * COLS], F8, kind="ExternalInput")
    # mask-matmul constants packed in one tensor: [mlo | mup | ident]
    mc_d = nc.dram_tensor("mconst", [128, 384], F8, kind="ExternalInput")
    rows_d = nc.dram_tensor("rows", [128, 8], F32, kind="ExternalOutput")
    cols_d = nc.dram_tensor("cols", [8, BAND], F32, kind="ExternalOutput")

    with tile.TileContext(nc) as tc:
        with (
            tc.tile_pool(name="big", bufs=1) as big,
            tc.tile_pool(name="ep", bufs=2) as ep,
            tc.tile_pool(name="cg", bufs=2) as cg,
            tc.tile_pool(name="small", bufs=1) as small,
            tc.tile_pool(name="ps", bufs=2, space="PSUM") as ps,
            tc.tile_pool(name="cs", bufs=2, space="PSUM") as cs,
        ):
            z8 = big.tile([128, 2 * COLS], F8, tag="z8")
            # Input DMAs: first halves of BOTH planes first (rt0 needs both),
            # spread across engine queues so the ~600ns issue costs overlap
            H = COLS // 2
            mconst = small.tile([128, 384], F8, tag="mconst")
            mask_lo = mconst[:, 0:128]
            mask_up = mconst[:, 128:256]
            ident = mconst[:, 256:384]
            nc.sync.dma_start(out=z8[:, 0:H], in_=z_d.ap()[:, 0:H])
            nc.scalar.dma_start(
                out=z8[:, COLS : COLS + H], in_=z_d.ap()[:, COLS : COLS + H]
            )
            nc.gpsimd.dma_start(out=mconst[:, :], in_=mc_d.ap()[:, :])
            nc.sync.dma_start(out=z8[:, H:COLS], in_=z_d.ap()[:, H:COLS])
            nc.scalar.dma_start(
                out=z8[:, COLS + H : 2 * COLS],
                in_=z_d.ap()[:, COLS + H : 2 * COLS],
            )

            # pair selector [128, 2, 128]: plane 0 -> row 0 (even row-tile),
            # plane 1 -> row 1 (odd row-tile); every pair's colsums land at
            # psum partitions 0:2 (engine partition-base must be 0/32/64/96).
            # Full 128-col stationary: narrower ones fail the LDW ISA check.
            self_f = small.tile([128, 256], F32, tag="selftmp")
            nc.vector.memset(self_f[:, :], 0.0)
            nc.vector.memset(self_f[:, 0:1], 1.0)
            nc.vector.memset(self_f[:, 129:130], 1.0)
            sel = small.tile([128, 256], F8, tag="sel")
            nc.vector.tensor_copy(sel[:, :], self_f[:, :])

            rowsums = small.tile([128, 24], F32, tag="rsum")
            zbase = z8[:, :]

            et = None
            for rt in range(8):
                if rt % 2 == 0:
                    et = ep.tile([128, 2 * BAND], F8, tag="e")
                pl = rt % 2
                ebase = et[:, :]
                for si, (off, w) in enumerate(SEGS):
                    pq = ps.tile([128, 1536], F32, tag="pq")
                    lhsT = _plane3(zbase, rt * 128, COLS, 128)
                    for sub0 in range(0, w, 512):
                        sw = min(512, w - sub0)
                        # the masked sub-tiles keep their accumulation group
                        # open for the mask matmul below
                        masked = (si == 0 and sub0 == 0) or (
                            si == 2 and sub0 == 1024
                        )
                        nc.tensor.matmul(
                            pq[:, sub0 : sub0 + sw],
                            lhsT,
                            _plane3(zbase, rt * 128 + off + sub0, COLS, sw),
                            start=True,
                            stop=not masked,
                            perf_mode=DR,
                        )
                    if si == 0:
                        # first band tile: mask distance <= 0 (diag+lower):
                        # pq[p,t] += NEG*1[t<=p] as a matmul (lhsT[k,m] =
                        # NEG*1[k<=m], rhs=I), keeping the mask on the PE so
                        # no other engine gates the exp
                        nc.tensor.matmul(
                            pq[:, 0:128],
                            mask_lo,
                            ident,
                            start=False,
                            stop=True,
                            skip_group_check=True,
                        )
                    if si == 2:
                        # wedge (distance ~4096): mask d > 4096
                        nc.tensor.matmul(
                            pq[:, 1024:1152],
                            mask_up,
                            ident,
                            start=False,
                            stop=True,
                            skip_group_check=True,
                        )
                    nc.scalar.activation(
                        et[:, pl * BAND + off : pl * BAND + off + w],
                        pq[:, 0:w],
                        AF.Exp,
                        scale=2.0,
                        accum_out=rowsums[:, rt * 3 + si : rt * 3 + si + 1],
                    )
                    if pl == 1:
                        # colsum this segment for the (rt-1, rt) pair as soon
                        # as both exps have been issued; DMA can't read PSUM,
                        # so bounce rows 0:2 through sbuf staging. On the very
                        # last segment ACT is done with exps, so split the
                        # copies between DVE and ACT to shorten the tail.
                        p = rt // 2
                        if si == 0:
                            cstage = cg.tile([2, BAND], F32, tag="cstage")
                        last = rt == 7 and si == 2
                        for ci, sub0 in enumerate(range(off, off + w, 512)):
                            sw = min(512, off + w - sub0)
                            cst = cs.tile([128, 512], F32, tag="cs")
                            nc.tensor.matmul(
                                cst[0:128, 0:sw],
                                _plane3(sel[:, :], 0, 128, 128),
                                _plane3(ebase, sub0, BAND, sw),
                                start=True,
                                stop=True,
                                perf_mode=DR,
                            )
                            if last and ci % 2 == 1:
                                nc.scalar.copy(
                                    cstage[0:2, sub0 : sub0 + sw],
                                    cst[0:2, 0:sw],
                                )
                            else:
                                nc.vector.tensor_copy(
                                    cstage[0:2, sub0 : sub0 + sw],
                                    cst[0:2, 0:sw],
                                )
                        nc.sync.dma_start(
                            out=cols_d.ap()[2 * p : 2 * p + 2, off : off + w],
                            in_=cstage[0:2, off : off + w],
                        )

            # reduce rowsums [128, 8x3] -> [128, 8] and ship
            rsum8 = small.tile([128, 8], F32, tag="rs8")
            nc.vector.tensor_reduce(
                rsum8[:, :],
                rowsums[:, :].rearrange("p (a b) -> p a b", b=3),
                axis=mybir.AxisListType.X,
                op=mybir.AluOpType.add,
            )
            nc.sync.dma_start(out=rows_d.ap()[:, :], in_=rsum8[:, :])

    _patch_act_table_loads(nc)
    nc.compile()
    return nc


def _act_set_id_with_exp(nc) -> int:
    from concourse.hw_specs import get_activation_tables

    tabs = get_activation_tables(nc.m.arch)
    for i, (name, fns) in enumerate(tabs.items()):
        if AF.Exp in fns:
            return i
    raise RuntimeError("no activation table set with Exp")


def _patch_act_table_loads(nc) -> None:
    # Load the exp table once up front instead of per-switch reloads.
    set_id = _act_set_id_with_exp(nc)

    def _single_act_table_load():
        for blk in nc.main_func.blocks:
            insts = list(blk.instructions)
            for i, ins in enumerate(insts):
                if isinstance(ins, mybir.InstActivation):
                    load = mybir.InstLoadActFuncSet(
                        name=nc.get_next_instruction_name(),
                        act_func_set_id=set_id,
                        ins=[],
                        outs=[],
                    )
                    load.engine = mybir.EngineType.Activation
                    insts.insert(i, load)
                    blk.instructions = insts
                    break

    nc.insert_act_table_loads = _single_act_table_load


def _get_nc() -> bass.Bass:
    if "nc" not in _CACHE:
        _CACHE["nc"] = _build_nc()
    return _CACHE["nc"]


def _masks():
    # lhsT constants for the mask matmuls (rhs = identity):
    # pq[p, t] += mlo[t, p], so mlo[k, m] = NEG iff k <= m masks t <= p
    # (keeps d >= 1); mup[k, m] = NEG iff k > m masks t > p (keeps d <= 4096)
    k = np.arange(128)[:, None]
    m = np.arange(128)[None, :]
    mlo = np.where(k <= m, NEG_MASK, 0.0)
    mup = np.where(k > m, NEG_MASK, 0.0)
    ident = np.eye(128)
    return np.concatenate([mlo, mup, ident], axis=1).astype(NP_F8)


def kernel(emb_i: np.ndarray, emb_j: np.ndarray) -> np.ndarray:
    global LAST_RESULTS
    z = np.concatenate(
        [np.asarray(emb_i, dtype=np.float32), np.asarray(emb_j, dtype=np.float32)],
        axis=0,
    )  # [8192, 256]
    z /= np.maximum(np.sqrt((z * z).sum(axis=1, keepdims=True)), 1e-12)
    z8 = z.astype(NP_F8)           # device values, exact
    z8f = z8.astype(np.float32)
    zt8 = np.ascontiguousarray(z8.T)  # [256, 8192] fp8
    mconst = _masks()

    in_maps = []
    for c in range(N_CORES):
        ztc = zt8 if c == 0 else np.roll(zt8, -c * ROWS, axis=1)
        ztc = ztc[:, :COLS]
        # DoubleRow plane layout: [128, 2*COLS], partition p = dims (p, p+128)
        buf = np.ascontiguousarray(
            np.concatenate([ztc[:128, :], ztc[128:, :]], axis=1)
        )
        in_maps.append({"z8": buf, "mconst": mconst})

    nc = _get_nc()
    LAST_RESULTS = run_bass_kernel_spmd(nc, in_maps, list(range(N_CORES)))

    den = np.zeros(TWO_N, dtype=np.float64)
    band_j = np.arange(BAND)
    for c in range(N_CORES):
        r = LAST_RESULTS.results[c]
        rows = np.asarray(r["rows"], dtype=np.float64)  # [128, 8] (p, rt)
        cols = np.asarray(r["cols"], dtype=np.float64)  # [8, BAND]
        den[c * ROWS : (c + 1) * ROWS] += rows.T.reshape(-1)
        for rt in range(8):
            g = (c * ROWS + rt * 128 + band_j) % TWO_N
            den[g] += cols[rt]

    idx = np.arange(TWO_N)
    pidx = (idx + N) % TWO_N
    # distance-4096 pairs were computed by both endpoints: subtract once,
    # using the same fp8 z the device saw
    pos8 = (z8f[idx] * z8f[pidx]).sum(axis=1, dtype=np.float64)
    den -= np.exp(2.0 * pos8)
    # the loss's positive term uses full-precision z
    pos = (z[idx].astype(np.float64) * z[pidx].astype(np.float64)).sum(axis=1)
    loss = np.mean(np.log(den) - 2.0 * pos)
    return np.array(loss, dtype=np.float32)


# revision 29
# speedup vs baseline: 1.7695x; 1.0293x over previous
"""NT-Xent (SimCLR) contrastive loss on 8 Trainium2 NeuronCores — v3 (fp8).

Symmetric half-band design as v2: exp(sim) is symmetric, so each global row i
only computes columns at circular distance d = j-i mod 2N in [1, 4096]; every
unordered pair lands on exactly one core except d == 4096 (the positive pair),
which lands on both and is corrected on the host.

v3 changes vs v2:
- z is normalized on the HOST (f32) and shipped as fp8e4m3 in a DoubleRow
  plane layout [128, 2, COLS] (partition p holds dims p and p+128). This
  deletes the whole on-device normalization pipeline (squares, norm matmuls,
  rsqrt chain, broadcast DMAs) that caused multi-us dependency bubbles, and
  cuts input DMA 4x.
- All matmuls run fp8 DoubleRow (0.5 cyc/row, K=256 in one pass): the sim
  matmul needs one instruction per 512 output cols, and the exp'd band tiles
  of two adjacent row-tiles are column-summed in one paired matmul.
- exp runs on ACT from [128,1536] PSUM tiles (3 per row-tile: 1536+1536+1152)
  with accum_out giving f32 row sums for free; e is written back as fp8 only
  for the colsum matmul. ACT is the bottleneck engine (~39 us busy/core).
- Column sums go per-pair straight from PSUM to DRAM (rows 2p,2p+1 of the
  [8, BAND] output), no DVE evacuation.

Host assembles den from f32 row sums + fp8-rounded col sums, subtracts the
double-counted positive exp, and takes mean(log(D) - 2*pos) with pos in f32.
"""

import sys

for _p in ("/opt/trn_rl_repo",):
    if _p not in sys.path:
        sys.path.insert(0, _p)

import ml_dtypes
import numpy as np

import concourse.bass as bass
import concourse.tile as tile
from concourse import bacc, mybir
from concourse.bass_utils import run_bass_kernel_spmd

F32 = mybir.dt.float32
F8 = mybir.dt.float8e4
AF = mybir.ActivationFunctionType
DR = mybir.MatmulPerfMode.DoubleRow
NP_F8 = ml_dtypes.float8_e4m3

N_CORES = 8
N = 4096
D = 256
TWO_N = 2 * N            # 8192 rows of sim
ROWS = TWO_N // N_CORES  # 1024 rows per core
COLS = 5120              # rotated columns staged per core
BAND = 4224              # band columns per 128-row tile (4096 + 128 wedge)
NEG_MASK = -128.0        # fp8-exact; exp(2*(sim-128)) underflows to 0
SEGS = ((0, 1536), (1536, 1536), (3072, 1152))  # band segments per row-tile

_CACHE = {}
LAST_RESULTS = None


def _plane3(base: bass.AP, off: int, plane_stride: int, w: int) -> bass.AP:
    """[128, 2, w] DoubleRow view of a plane-major [128, 2*S] sbuf tile."""
    return bass.AP(
        tensor=base.tensor,
        offset=base.offset + off,
        ap=[list(base.ap[0]), [plane_stride, 2], [1, w]],
    )


def _build_nc() -> bass.Bass:
    nc = bacc.Bacc("TRN2", num_devices=N_CORES)

    z_d = nc.dram_tensor("z8", [128, 2 * COLS], F8, kind="ExternalInput")
    # mask-matmul constants packed in one tensor: [mlo | mup | ident]
    mc_d = nc.dram_tensor("mconst", [128, 384], F8, kind="ExternalInput")
    rows_d = nc.dram_tensor("rows", [128, 8], F32, kind="ExternalOutput")
    cols_d = nc.dram_tensor("cols", [8, BAND], F32, kind="ExternalOutput")

    with tile.TileContext(nc) as tc:
        with (
            tc.tile_pool(name="big", bufs=1) as big,
            tc.tile_pool(name="ep", bufs=2) as ep,
            tc.tile_pool(name="cg", bufs=2) as cg,
            tc.tile_pool(name="small", bufs=1) as small,
            tc.tile_pool(name="ps", bufs=2, space="PSUM") as ps,
            tc.tile_pool(name="cs", bufs=2, space="PSUM") as cs,
        ):
            z8 = big.tile([128, 2 * COLS], F8, tag="z8")
            # Input DMAs: first halves of BOTH planes first (rt0 needs both),
            # spread across engine queues so the ~600ns issue costs overlap
            H = COLS // 2
            mconst = small.tile([128, 384], F8, tag="mconst")
            mask_lo = mconst[:, 0:128]
            mask_up = mconst[:, 128:256]
            ident = mconst[:, 256:384]
            # 3 chunks per plane: [0:1664] covers rt0's first segment, so
            # compute starts after ~1/3 of the load; later chunks land before
            # the row-tiles that need them
            for a, b in ((0, 1664), (1664, 3392), (3392, COLS)):
                nc.sync.dma_start(out=z8[:, a:b], in_=z_d.ap()[:, a:b])
                nc.scalar.dma_start(
                    out=z8[:, COLS + a : COLS + b],
                    in_=z_d.ap()[:, COLS + a : COLS + b],
                )
                if a == 0:
                    nc.gpsimd.dma_start(out=mconst[:, :], in_=mc_d.ap()[:, :])

            # pair selector [128, 2, 128]: plane 0 -> row 0 (even row-tile),
            # plane 1 -> row 1 (odd row-tile); every pair's colsums land at
            # psum partitions 0:2 (engine partition-base must be 0/32/64/96).
            # Full 128-col stationary: narrower ones fail the LDW ISA check.
            self_f = small.tile([128, 256], F32, tag="selftmp")
            nc.vector.memset(self_f[:, :], 0.0)
            nc.vector.memset(self_f[:, 0:1], 1.0)
            nc.vector.memset(self_f[:, 129:130], 1.0)
            sel = small.tile([128, 256], F8, tag="sel")
            nc.vector.tensor_copy(sel[:, :], self_f[:, :])

            rowsums = small.tile([128, 24], F32, tag="rsum")
            zbase = z8[:, :]

            rsum8 = small.tile([128, 8], F32, tag="rs8")
            et = None
            for rt in range(8):
                if rt == 7:
                    # rowsums only lack rt7's segments; queue the reduce and
                    # its DMA (scalar queue) now so they overlap the final
                    # colsum chain instead of trailing it
                    nc.vector.tensor_reduce(
                        rsum8[:, 0:7],
                        rowsums[:, 0:21].rearrange("p (a b) -> p a b", b=3),
                        axis=mybir.AxisListType.X,
                        op=mybir.AluOpType.add,
                    )
                if rt % 2 == 0:
                    et = ep.tile([128, 2 * BAND], F8, tag="e")
                pl = rt % 2
                ebase = et[:, :]
                for si, (off, w) in enumerate(SEGS):
                    pq = ps.tile([128, 1536], F32, tag="pq")
                    lhsT = _plane3(zbase, rt * 128, COLS, 128)
                    for sub0 in range(0, w, 512):
                        sw = min(512, w - sub0)
                        # the masked sub-tiles keep their accumulation group
                        # open for the mask matmul below
                        masked = (si == 0 and sub0 == 0) or (
                            si == 2 and sub0 == 1024
                        )
                        nc.tensor.matmul(
                            pq[:, sub0 : sub0 + sw],
                            lhsT,
                            _plane3(zbase, rt * 128 + off + sub0, COLS, sw),
                            start=True,
                            stop=not masked,
                            perf_mode=DR,
                        )
                    if si == 0:
                        # first band tile: mask distance <= 0 (diag+lower):
                        # pq[p,t] += NEG*1[t<=p] as a matmul (lhsT[k,m] =
                        # NEG*1[k<=m], rhs=I), keeping the mask on the PE so
                        # no other engine gates the exp
                        nc.tensor.matmul(
                            pq[:, 0:128],
                            mask_lo,
                            ident,
                            start=False,
                            stop=True,
                            skip_group_check=True,
                        )
                    if si == 2:
                        # wedge (distance ~4096): mask d > 4096
                        nc.tensor.matmul(
                            pq[:, 1024:1152],
                            mask_up,
                            ident,
                            start=False,
                            stop=True,
                            skip_group_check=True,
                        )
                    nc.scalar.activation(
                        et[:, pl * BAND + off : pl * BAND + off + w],
                        pq[:, 0:w],
                        AF.Exp,
                        scale=2.0,
                        accum_out=rowsums[:, rt * 3 + si : rt * 3 + si + 1],
                    )
                    if pl == 1:
                        # colsum this segment for the (rt-1, rt) pair as soon
                        # as both exps have been issued; DMA can't read PSUM,
                        # so bounce rows 0:2 through sbuf staging. On the very
                        # last segment ACT is done with exps, so split the
                        # copies between DVE and ACT to shorten the tail.
                        p = rt // 2
                        if si == 0:
                            cstage = cg.tile([2, BAND], F32, tag="cstage")
                        last = rt == 7 and si == 2
                        for ci, sub0 in enumerate(range(off, off + w, 512)):
                            sw = min(512, off + w - sub0)
                            cst = cs.tile([128, 512], F32, tag="cs")
                            nc.tensor.matmul(
                                cst[0:128, 0:sw],
                                _plane3(sel[:, :], 0, 128, 128),
                                _plane3(ebase, sub0, BAND, sw),
                                start=True,
                                stop=True,
                                perf_mode=DR,
                            )
                            if last and ci % 2 == 1:
                                nc.scalar.copy(
                                    cstage[0:2, sub0 : sub0 + sw],
                                    cst[0:2, 0:sw],
                                )
                            else:
                                nc.vector.tensor_copy(
                                    cstage[0:2, sub0 : sub0 + sw],
                                    cst[0:2, 0:sw],
                                )
                        nc.sync.dma_start(
                            out=cols_d.ap()[2 * p : 2 * p + 2, off : off + w],
                            in_=cstage[0:2, off : off + w],
                        )

            # rt7's rowsums column: reduced separately so the first 7 could
            # be reduced early; ship on the idle scalar queue
            nc.vector.tensor_reduce(
                rsum8[:, 7:8],
                rowsums[:, 21:24].rearrange("p (a b) -> p a b", b=3),
                axis=mybir.AxisListType.X,
                op=mybir.AluOpType.add,
            )
            nc.scalar.dma_start(out=rows_d.ap()[:, :], in_=rsum8[:, :])

    _patch_act_table_loads(nc)
    nc.compile()
    return nc


def _act_set_id_with_exp(nc) -> int:
    from concourse.hw_specs import get_activation_tables

    tabs = get_activation_tables(nc.m.arch)
    for i, (name, fns) in enumerate(tabs.items()):
        if AF.Exp in fns:
            return i
    raise RuntimeError("no activation table set with Exp")


def _patch_act_table_loads(nc) -> None:
    # Load the exp table once up front instead of per-switch reloads.
    set_id = _act_set_id_with_exp(nc)

    def _single_act_table_load():
        for blk in nc.main_func.blocks:
            insts = list(blk.instructions)
            for i, ins in enumerate(insts):
                if isinstance(ins, mybir.InstActivation):
                    load = mybir.InstLoadActFuncSet(
                        name=nc.get_next_instruction_name(),
                        act_func_set_id=set_id,
                        ins=[],
                        outs=[],
                    )
                    load.engine = mybir.EngineType.Activation
                    insts.insert(i, load)
                    blk.instructions = insts
                    break

    nc.insert_act_table_loads = _single_act_table_load


def _get_nc() -> bass.Bass:
    if "nc" not in _CACHE:
        _CACHE["nc"] = _build_nc()
    return _CACHE["nc"]


def _masks():
    # lhsT constants for the mask matmuls (rhs = identity):
    # pq[p, t] += mlo[t, p], so mlo[k, m] = NEG iff k <= m masks t <= p
    # (keeps d >= 1); mup[k, m] = NEG iff k > m masks t > p (keeps d <= 4096)
    k = np.arange(128)[:, None]
    m = np.arange(128)[None, :]
    mlo = np.where(k <= m, NEG_MASK, 0.0)
    mup = np.where(k > m, NEG_MASK, 0.0)
    ident = np.eye(128)
    return np.concatenate([mlo, mup, ident], axis=1).astype(NP_F8)


def kernel(emb_i: np.ndarray, emb_j: np.ndarray) -> np.ndarray:
    global LAST_RESULTS
    z = np.concatenate(
        [np.asarray(emb_i, dtype=np.float32), np.asarray(emb_j, dtype=np.float32)],
        axis=0,
    )  # [8192, 256]
    z /= np.maximum(np.sqrt((z * z).sum(axis=1, keepdims=True)), 1e-12)
    z8 = z.astype(NP_F8)           # device values, exact
    z8f = z8.astype(np.float32)
    zt8 = np.ascontiguousarray(z8.T)  # [256, 8192] fp8
    mconst = _masks()

    in_maps = []
    for c in range(N_CORES):
        ztc = zt8 if c == 0 else np.roll(zt8, -c * ROWS, axis=1)
        ztc = ztc[:, :COLS]
        # DoubleRow plane layout: [128, 2*COLS], partition p = dims (p, p+128)
        buf = np.ascontiguousarray(
            np.concatenate([ztc[:128, :], ztc[128:, :]], axis=1)
        )
        in_maps.append({"z8": buf, "mconst": mconst})

    nc = _get_nc()
    LAST_RESULTS = run_bass_kernel_spmd(nc, in_maps, list(range(N_CORES)))

    den = np.zeros(TWO_N, dtype=np.float64)
    band_j = np.arange(BAND)
    for c in range(N_CORES):
        r = LAST_RESULTS.results[c]
        rows = np.asarray(r["rows"], dtype=np.float64)  # [128, 8] (p, rt)
        cols = np.asarray(r["cols"], dtype=np.float64)  # [8, BAND]
        den[c * ROWS : (c + 1) * ROWS] += rows.T.reshape(-1)
        for rt in range(8):
            g = (c * ROWS + rt * 128 + band_j) % TWO_N
            den[g] += cols[rt]

    idx = np.arange(TWO_N)
    pidx = (idx + N) % TWO_N
    # distance-4096 pairs were computed by both endpoints: subtract once,
    # using the same fp8 z the device saw
    pos8 = (z8f[idx] * z8f[pidx]).sum(axis=1, dtype=np.float64)
    den -= np.exp(2.0 * pos8)
    # the loss's positive term uses full-precision z
    pos = (z[idx].astype(np.float64) * z[pidx].astype(np.float64)).sum(axis=1)
    loss = np.mean(np.log(den) - 2.0 * pos)
    return np.array(loss, dtype=np.float32)


# revision 33
# speedup vs baseline: 1.7759x; 1.0036x over previous
"""NT-Xent (SimCLR) contrastive loss on 8 Trainium2 NeuronCores — v3 (fp8).

Symmetric half-band design as v2: exp(sim) is symmetric, so each global row i
only computes columns at circular distance d = j-i mod 2N in [1, 4096]; every
unordered pair lands on exactly one core except d == 4096 (the positive pair),
which lands on both and is corrected on the host.

v3 changes vs v2:
- z is normalized on the HOST (f32) and shipped as fp8e4m3 in a DoubleRow
  plane layout [128, 2, COLS] (partition p holds dims p and p+128). This
  deletes the whole on-device normalization pipeline (squares, norm matmuls,
  rsqrt chain, broadcast DMAs) that caused multi-us dependency bubbles, and
  cuts input DMA 4x.
- All matmuls run fp8 DoubleRow (0.5 cyc/row, K=256 in one pass): the sim
  matmul needs one instruction per 512 output cols, and the exp'd band tiles
  of two adjacent row-tiles are column-summed in one paired matmul.
- exp runs on ACT from [128,1536] PSUM tiles (3 per row-tile: 1536+1536+1152)
  with accum_out giving f32 row sums for free; e is written back as fp8 only
  for the colsum matmul. ACT is the bottleneck engine (~39 us busy/core).
- Column sums go per-pair straight from PSUM to DRAM (rows 2p,2p+1 of the
  [8, BAND] output), no DVE evacuation.

Host assembles den from f32 row sums + fp8-rounded col sums, subtracts the
double-counted positive exp, and takes mean(log(D) - 2*pos) with pos in f32.
"""

import sys

for _p in ("/opt/trn_rl_repo",):
    if _p not in sys.path:
        sys.path.insert(0, _p)

import ml_dtypes
import numpy as np

import concourse.bass as bass
import concourse.tile as tile
from concourse import bacc, mybir
from concourse.bass_utils import run_bass_kernel_spmd

F32 = mybir.dt.float32
F8 = mybir.dt.float8e4
AF = mybir.ActivationFunctionType
DR = mybir.MatmulPerfMode.DoubleRow
NP_F8 = ml_dtypes.float8_e4m3

N_CORES = 8
N = 4096
D = 256
TWO_N = 2 * N            # 8192 rows of sim
ROWS = TWO_N // N_CORES  # 1024 rows per core
COLS = 5120              # rotated columns staged per core
BAND = 4224              # band columns per 128-row tile (4096 + 128 wedge)
NEG_MASK = -128.0        # fp8-exact; exp(2*(sim-128)) underflows to 0
SEGS = ((0, 1536), (1536, 1536), (3072, 1152))  # band segments per row-tile

_CACHE = {}
LAST_RESULTS = None


def _plane3(base: bass.AP, off: int, plane_stride: int, w: int) -> bass.AP:
    """[128, 2, w] DoubleRow view of a plane-major [128, 2*S] sbuf tile."""
    return bass.AP(
        tensor=base.tensor,
        offset=base.offset + off,
        ap=[list(base.ap[0]), [plane_stride, 2], [1, w]],
    )


def _build_nc() -> bass.Bass:
    nc = bacc.Bacc("TRN2", num_devices=N_CORES)

    z_d = nc.dram_tensor("z8", [128, 2 * COLS], F8, kind="ExternalInput")
    # mask-matmul constants packed in one tensor: [mlo | mup | ident]
    mc_d = nc.dram_tensor("mconst", [128, 384], F8, kind="ExternalInput")
    rows_d = nc.dram_tensor("rows", [128, 8], F32, kind="ExternalOutput")
    cols_d = nc.dram_tensor("cols", [8, BAND], F32, kind="ExternalOutput")

    with tile.TileContext(nc) as tc:
        with (
            tc.tile_pool(name="big", bufs=1) as big,
            tc.tile_pool(name="ep", bufs=2) as ep,
            tc.tile_pool(name="cg", bufs=2) as cg,
            tc.tile_pool(name="small", bufs=1) as small,
            tc.tile_pool(name="ps", bufs=2, space="PSUM") as ps,
            tc.tile_pool(name="cs", bufs=2, space="PSUM") as cs,
        ):
            z8 = big.tile([128, 2 * COLS], F8, tag="z8")
            # Input DMAs: first halves of BOTH planes first (rt0 needs both),
            # spread across engine queues so the ~600ns issue costs overlap
            H = COLS // 2
            mconst = small.tile([128, 384], F8, tag="mconst")
            mask_lo = mconst[:, 0:128]
            mask_up = mconst[:, 128:256]
            ident = mconst[:, 256:384]
            # 3 chunks per plane: [0:1664] covers rt0's first segment, so
            # compute starts after ~1/3 of the load; later chunks land before
            # the row-tiles that need them
            for a, b in ((0, 1664), (1664, 3392), (3392, COLS)):
                nc.sync.dma_start(out=z8[:, a:b], in_=z_d.ap()[:, a:b])
                nc.scalar.dma_start(
                    out=z8[:, COLS + a : COLS + b],
                    in_=z_d.ap()[:, COLS + a : COLS + b],
                )
                if a == 0:
                    nc.gpsimd.dma_start(out=mconst[:, :], in_=mc_d.ap()[:, :])

            # pair selector [128, 2, 128]: plane 0 -> row 0 (even row-tile),
            # plane 1 -> row 1 (odd row-tile); every pair's colsums land at
            # psum partitions 0:2 (engine partition-base must be 0/32/64/96).
            # Full 128-col stationary: narrower ones fail the LDW ISA check.
            self_f = small.tile([128, 256], F32, tag="selftmp")
            nc.vector.memset(self_f[:, :], 0.0)
            nc.vector.memset(self_f[:, 0:1], 1.0)
            nc.vector.memset(self_f[:, 129:130], 1.0)
            sel = small.tile([128, 256], F8, tag="sel")
            nc.vector.tensor_copy(sel[:, :], self_f[:, :])

            rowsums = small.tile([128, 24], F32, tag="rsum")
            zbase = z8[:, :]

            rsum8 = small.tile([128, 8], F32, tag="rs8")

            # colsum emission is deferred by one segment: placed directly
            # after the NEXT segment's matmuls in the PE stream, the colsum
            # (which waits on its segment's exps) no longer head-of-line
            # blocks the pq fill the ACT pipeline needs next.
            state = {"cstage": None, "pending": None}

            def emit_colsum(p, et_pair, off, w, last):
                if off == 0:
                    cstage_t = cg.tile([2, BAND], F32, tag="cstage")
                    state["cstage"] = cstage_t
                cstage = state["cstage"]
                eb = et_pair[:, :]
                for ci, sub0 in enumerate(range(off, off + w, 512)):
                    sw = min(512, off + w - sub0)
                    cst = cs.tile([128, 512], F32, tag="cs")
                    nc.tensor.matmul(
                        cst[0:128, 0:sw],
                        _plane3(sel[:, :], 0, 128, 128),
                        _plane3(eb, sub0, BAND, sw),
                        start=True,
                        stop=True,
                        perf_mode=DR,
                    )
                    if last and ci % 2 == 1:
                        # ACT is done with exps by now; split the tail copies
                        # across ACT and DVE
                        nc.scalar.copy(
                            cstage[0:2, sub0 : sub0 + sw], cst[0:2, 0:sw]
                        )
                    else:
                        nc.vector.tensor_copy(
                            cstage[0:2, sub0 : sub0 + sw], cst[0:2, 0:sw]
                        )
                nc.sync.dma_start(
                    out=cols_d.ap()[2 * p : 2 * p + 2, off : off + w],
                    in_=cstage[0:2, off : off + w],
                )

            et = None
            for rt in range(8):
                if rt == 7:
                    # rowsums only lack rt7's segments; queue the reduce and
                    # its DMA (scalar queue) now so they overlap the final
                    # colsum chain instead of trailing it
                    nc.vector.tensor_reduce(
                        rsum8[:, 0:7],
                        rowsums[:, 0:21].rearrange("p (a b) -> p a b", b=3),
                        axis=mybir.AxisListType.X,
                        op=mybir.AluOpType.add,
                    )
                if rt % 2 == 0:
                    et = ep.tile([128, 2 * BAND], F8, tag="e")
                pl = rt % 2
                ebase = et[:, :]
                for si, (off, w) in enumerate(SEGS):
                    pq = ps.tile([128, 1536], F32, tag="pq")
                    lhsT = _plane3(zbase, rt * 128, COLS, 128)
                    for sub0 in range(0, w, 512):
                        sw = min(512, w - sub0)
                        # the masked sub-tiles keep their accumulation group
                        # open for the mask matmul below
                        masked = (si == 0 and sub0 == 0) or (
                            si == 2 and sub0 == 1024
                        )
                        nc.tensor.matmul(
                            pq[:, sub0 : sub0 + sw],
                            lhsT,
                            _plane3(zbase, rt * 128 + off + sub0, COLS, sw),
                            start=True,
                            stop=not masked,
                            perf_mode=DR,
                        )
                    if si == 0:
                        # first band tile: mask distance <= 0 (diag+lower):
                        # pq[p,t] += NEG*1[t<=p] as a matmul (lhsT[k,m] =
                        # NEG*1[k<=m], rhs=I), keeping the mask on the PE so
                        # no other engine gates the exp
                        nc.tensor.matmul(
                            pq[:, 0:128],
                            mask_lo,
                            ident,
                            start=False,
                            stop=True,
                            skip_group_check=True,
                        )
                    if si == 2:
                        # wedge (distance ~4096): mask d > 4096
                        nc.tensor.matmul(
                            pq[:, 1024:1152],
                            mask_up,
                            ident,
                            start=False,
                            stop=True,
                            skip_group_check=True,
                        )
                    nc.scalar.activation(
                        et[:, pl * BAND + off : pl * BAND + off + w],
                        pq[:, 0:w],
                        AF.Exp,
                        scale=2.0,
                        accum_out=rowsums[:, rt * 3 + si : rt * 3 + si + 1],
                    )
                    if state["pending"] is not None:
                        args = state["pending"]
                        state["pending"] = None
                        emit_colsum(*args)
                    if pl == 1:
                        state["pending"] = (rt // 2, et, off, w, False)

            # final pair's last colsum group trails the last exp
            p, et_pair, off, w, _ = state["pending"]
            state["pending"] = None
            emit_colsum(p, et_pair, off, w, True)

            # rt7's rowsums column: reduced separately so the first 7 could
            # be reduced early; ship on the idle scalar queue
            nc.vector.tensor_reduce(
                rsum8[:, 7:8],
                rowsums[:, 21:24].rearrange("p (a b) -> p a b", b=3),
                axis=mybir.AxisListType.X,
                op=mybir.AluOpType.add,
            )
            nc.scalar.dma_start(out=rows_d.ap()[:, :], in_=rsum8[:, :])

    _patch_act_table_loads(nc)
    nc.compile()
    return nc


def _act_set_id_with_exp(nc) -> int:
    from concourse.hw_specs import get_activation_tables

    tabs = get_activation_tables(nc.m.arch)
    for i, (name, fns) in enumerate(tabs.items()):
        if AF.Exp in fns:
            return i
    raise RuntimeError("no activation table set with Exp")


def _patch_act_table_loads(nc) -> None:
    # Load the exp table once up front instead of per-switch reloads.
    set_id = _act_set_id_with_exp(nc)

    def _single_act_table_load():
        for blk in nc.main_func.blocks:
            insts = list(blk.instructions)
            for i, ins in enumerate(insts):
                if isinstance(ins, mybir.InstActivation):
                    load = mybir.InstLoadActFuncSet(
                        name=nc.get_next_instruction_name(),
                        act_func_set_id=set_id,
                        ins=[],
                        outs=[],
                    )
                    load.engine = mybir.EngineType.Activation
                    insts.insert(i, load)
                    blk.instructions = insts
                    break

    nc.insert_act_table_loads = _single_act_table_load


def _get_nc() -> bass.Bass:
    if "nc" not in _CACHE:
        _CACHE["nc"] = _build_nc()
    return _CACHE["nc"]


def _masks():
    # lhsT constants for the mask matmuls (rhs = identity):
    # pq[p, t] += mlo[t, p], so mlo[k, m] = NEG iff k <= m masks t <= p
    # (keeps d >= 1); mup[k, m] = NEG iff k > m masks t > p (keeps d <= 4096)
    k = np.arange(128)[:, None]
    m = np.arange(128)[None, :]
    mlo = np.where(k <= m, NEG_MASK, 0.0)
    mup = np.where(k > m, NEG_MASK, 0.0)
    ident = np.eye(128)
    return np.concatenate([mlo, mup, ident], axis=1).astype(NP_F8)


def kernel(emb_i: np.ndarray, emb_j: np.ndarray) -> np.ndarray:
    global LAST_RESULTS
    z = np.concatenate(
        [np.asarray(emb_i, dtype=np.float32), np.asarray(emb_j, dtype=np.float32)],
        axis=0,
    )  # [8192, 256]
    z /= np.maximum(np.sqrt((z * z).sum(axis=1, keepdims=True)), 1e-12)
    z8 = z.astype(NP_F8)           # device values, exact
    z8f = z8.astype(np.float32)
    zt8 = np.ascontiguousarray(z8.T)  # [256, 8192] fp8
    mconst = _masks()

    in_maps = []
    for c in range(N_CORES):
        ztc = zt8 if c == 0 else np.roll(zt8, -c * ROWS, axis=1)
        ztc = ztc[:, :COLS]
        # DoubleRow plane layout: [128, 2*COLS], partition p = dims (p, p+128)
        buf = np.ascontiguousarray(
            np.concatenate([ztc[:128, :], ztc[128:, :]], axis=1)
        )
        in_maps.append({"z8": buf, "mconst": mconst})

    nc = _get_nc()
    LAST_RESULTS = run_bass_kernel_spmd(nc, in_maps, list(range(N_CORES)))

    den = np.zeros(TWO_N, dtype=np.float64)
    band_j = np.arange(BAND)
    for c in range(N_CORES):
        r = LAST_RESULTS.results[c]
        rows = np.asarray(r["rows"], dtype=np.float64)  # [128, 8] (p, rt)
        cols = np.asarray(r["cols"], dtype=np.float64)  # [8, BAND]
        den[c * ROWS : (c + 1) * ROWS] += rows.T.reshape(-1)
        for rt in range(8):
            g = (c * ROWS + rt * 128 + band_j) % TWO_N
            den[g] += cols[rt]

    idx = np.arange(TWO_N)
    pidx = (idx + N) % TWO_N
    # distance-4096 pairs were computed by both endpoints: subtract once,
    # using the same fp8 z the device saw
    pos8 = (z8f[idx] * z8f[pidx]).sum(axis=1, dtype=np.float64)
    den -= np.exp(2.0 * pos8)
    # the loss's positive term uses full-precision z
    pos = (z[idx].astype(np.float64) * z[pidx].astype(np.float64)).sum(axis=1)
    loss = np.mean(np.log(den) - 2.0 * pos)
    return np.array(loss, dtype=np.float32)


# revision 34
# speedup vs baseline: 1.7766x; 1.0003x over previous
"""NT-Xent (SimCLR) contrastive loss on 8 Trainium2 NeuronCores — v3 (fp8).

Symmetric half-band design as v2: exp(sim) is symmetric, so each global row i
only computes columns at circular distance d = j-i mod 2N in [1, 4096]; every
unordered pair lands on exactly one core except d == 4096 (the positive pair),
which lands on both and is corrected on the host.

v3 changes vs v2:
- z is normalized on the HOST (f32) and shipped as fp8e4m3 in a DoubleRow
  plane layout [128, 2, COLS] (partition p holds dims p and p+128). This
  deletes the whole on-device normalization pipeline (squares, norm matmuls,
  rsqrt chain, broadcast DMAs) that caused multi-us dependency bubbles, and
  cuts input DMA 4x.
- All matmuls run fp8 DoubleRow (0.5 cyc/row, K=256 in one pass): the sim
  matmul needs one instruction per 512 output cols, and the exp'd band tiles
  of two adjacent row-tiles are column-summed in one paired matmul.
- exp runs on ACT from [128,1536] PSUM tiles (3 per row-tile: 1536+1536+1152)
  with accum_out giving f32 row sums for free; e is written back as fp8 only
  for the colsum matmul. ACT is the bottleneck engine (~39 us busy/core).
- Column sums go per-pair straight from PSUM to DRAM (rows 2p,2p+1 of the
  [8, BAND] output), no DVE evacuation.

Host assembles den from f32 row sums + fp8-rounded col sums, subtracts the
double-counted positive exp, and takes mean(log(D) - 2*pos) with pos in f32.
"""

import sys

for _p in ("/opt/trn_rl_repo",):
    if _p not in sys.path:
        sys.path.insert(0, _p)

import ml_dtypes
import numpy as np

import concourse.bass as bass
import concourse.tile as tile
from concourse import bacc, mybir
from concourse.bass_utils import run_bass_kernel_spmd

F32 = mybir.dt.float32
F8 = mybir.dt.float8e4
AF = mybir.ActivationFunctionType
DR = mybir.MatmulPerfMode.DoubleRow
NP_F8 = ml_dtypes.float8_e4m3

N_CORES = 8
N = 4096
D = 256
TWO_N = 2 * N            # 8192 rows of sim
ROWS = TWO_N // N_CORES  # 1024 rows per core
COLS = 5120              # rotated columns staged per core
BAND = 4224              # band columns per 128-row tile (4096 + 128 wedge)
NEG_MASK = -128.0        # fp8-exact; exp(2*(sim-128)) underflows to 0
SEGS = ((0, 1536), (1536, 1536), (3072, 1152))  # band segments per row-tile

_CACHE = {}
LAST_RESULTS = None


def _plane3(base: bass.AP, off: int, plane_stride: int, w: int) -> bass.AP:
    """[128, 2, w] DoubleRow view of a plane-major [128, 2*S] sbuf tile."""
    return bass.AP(
        tensor=base.tensor,
        offset=base.offset + off,
        ap=[list(base.ap[0]), [plane_stride, 2], [1, w]],
    )


def _build_nc() -> bass.Bass:
    nc = bacc.Bacc("TRN2", num_devices=N_CORES)

    z_d = nc.dram_tensor("z8", [128, 2 * COLS], F8, kind="ExternalInput")
    # mask-matmul constants packed in one tensor: [mlo | mup | ident]
    mc_d = nc.dram_tensor("mconst", [128, 384], F8, kind="ExternalInput")
    rows_d = nc.dram_tensor("rows", [128, 8], F32, kind="ExternalOutput")
    cols_d = nc.dram_tensor("cols", [8, BAND], F32, kind="ExternalOutput")

    with tile.TileContext(nc) as tc:
        with (
            tc.tile_pool(name="big", bufs=1) as big,
            tc.tile_pool(name="ep", bufs=2) as ep,
            tc.tile_pool(name="cg", bufs=2) as cg,
            tc.tile_pool(name="small", bufs=1) as small,
            tc.tile_pool(name="ps", bufs=2, space="PSUM") as ps,
            tc.tile_pool(name="cs", bufs=2, space="PSUM") as cs,
        ):
            z8 = big.tile([128, 2 * COLS], F8, tag="z8")
            # Input DMAs: first halves of BOTH planes first (rt0 needs both),
            # spread across engine queues so the ~600ns issue costs overlap
            H = COLS // 2
            mconst = small.tile([128, 384], F8, tag="mconst")
            mask_lo = mconst[:, 0:128]
            mask_up = mconst[:, 128:256]
            ident = mconst[:, 256:384]
            # 4 chunks per plane: [0:640] lets rt0's first matmul start after
            # ~8% of the load and overlap the rest; later chunks land before
            # the row-tiles that need them
            for a, b in ((0, 640), (640, 1664), (1664, 3392), (3392, COLS)):
                nc.sync.dma_start(out=z8[:, a:b], in_=z_d.ap()[:, a:b])
                nc.scalar.dma_start(
                    out=z8[:, COLS + a : COLS + b],
                    in_=z_d.ap()[:, COLS + a : COLS + b],
                )
                if a == 0:
                    nc.gpsimd.dma_start(out=mconst[:, :], in_=mc_d.ap()[:, :])

            # pair selector [128, 2, 128]: plane 0 -> row 0 (even row-tile),
            # plane 1 -> row 1 (odd row-tile); every pair's colsums land at
            # psum partitions 0:2 (engine partition-base must be 0/32/64/96).
            # Full 128-col stationary: narrower ones fail the LDW ISA check.
            self_f = small.tile([128, 256], F32, tag="selftmp")
            nc.vector.memset(self_f[:, :], 0.0)
            nc.vector.memset(self_f[:, 0:1], 1.0)
            nc.vector.memset(self_f[:, 129:130], 1.0)
            sel = small.tile([128, 256], F8, tag="sel")
            nc.vector.tensor_copy(sel[:, :], self_f[:, :])

            rowsums = small.tile([128, 24], F32, tag="rsum")
            zbase = z8[:, :]

            rsum8 = small.tile([128, 8], F32, tag="rs8")

            # colsum emission is deferred by one segment: placed directly
            # after the NEXT segment's matmuls in the PE stream, the colsum
            # (which waits on its segment's exps) no longer head-of-line
            # blocks the pq fill the ACT pipeline needs next.
            state = {"cstage": None, "pending": None}

            def emit_colsum(p, et_pair, off, w, last):
                if off == 0:
                    cstage_t = cg.tile([2, BAND], F32, tag="cstage")
                    state["cstage"] = cstage_t
                cstage = state["cstage"]
                eb = et_pair[:, :]
                for ci, sub0 in enumerate(range(off, off + w, 512)):
                    sw = min(512, off + w - sub0)
                    cst = cs.tile([128, 512], F32, tag="cs")
                    nc.tensor.matmul(
                        cst[0:128, 0:sw],
                        _plane3(sel[:, :], 0, 128, 128),
                        _plane3(eb, sub0, BAND, sw),
                        start=True,
                        stop=True,
                        perf_mode=DR,
                    )
                    if last and ci % 2 == 1:
                        # ACT is done with exps by now; split the tail copies
                        # across ACT and DVE
                        nc.scalar.copy(
                            cstage[0:2, sub0 : sub0 + sw], cst[0:2, 0:sw]
                        )
                    else:
                        nc.vector.tensor_copy(
                            cstage[0:2, sub0 : sub0 + sw], cst[0:2, 0:sw]
                        )
                nc.sync.dma_start(
                    out=cols_d.ap()[2 * p : 2 * p + 2, off : off + w],
                    in_=cstage[0:2, off : off + w],
                )

            et = None
            for rt in range(8):
                if rt == 7:
                    # rowsums only lack rt7's segments; queue the reduce and
                    # its DMA (scalar queue) now so they overlap the final
                    # colsum chain instead of trailing it
                    nc.vector.tensor_reduce(
                        rsum8[:, 0:7],
                        rowsums[:, 0:21].rearrange("p (a b) -> p a b", b=3),
                        axis=mybir.AxisListType.X,
                        op=mybir.AluOpType.add,
                    )
                if rt % 2 == 0:
                    et = ep.tile([128, 2 * BAND], F8, tag="e")
                pl = rt % 2
                ebase = et[:, :]
                for si, (off, w) in enumerate(SEGS):
                    pq = ps.tile([128, 1536], F32, tag="pq")
                    lhsT = _plane3(zbase, rt * 128, COLS, 128)
                    for sub0 in range(0, w, 512):
                        sw = min(512, w - sub0)
                        # the masked sub-tiles keep their accumulation group
                        # open for the mask matmul below
                        masked = (si == 0 and sub0 == 0) or (
                            si == 2 and sub0 == 1024
                        )
                        nc.tensor.matmul(
                            pq[:, sub0 : sub0 + sw],
                            lhsT,
                            _plane3(zbase, rt * 128 + off + sub0, COLS, sw),
                            start=True,
                            stop=not masked,
                            perf_mode=DR,
                        )
                    if si == 0:
                        # first band tile: mask distance <= 0 (diag+lower):
                        # pq[p,t] += NEG*1[t<=p] as a matmul (lhsT[k,m] =
                        # NEG*1[k<=m], rhs=I), keeping the mask on the PE so
                        # no other engine gates the exp
                        nc.tensor.matmul(
                            pq[:, 0:128],
                            mask_lo,
                            ident,
                            start=False,
                            stop=True,
                            skip_group_check=True,
                        )
                    if si == 2:
                        # wedge (distance ~4096): mask d > 4096
                        nc.tensor.matmul(
                            pq[:, 1024:1152],
                            mask_up,
                            ident,
                            start=False,
                            stop=True,
                            skip_group_check=True,
                        )
                    nc.scalar.activation(
                        et[:, pl * BAND + off : pl * BAND + off + w],
                        pq[:, 0:w],
                        AF.Exp,
                        scale=2.0,
                        accum_out=rowsums[:, rt * 3 + si : rt * 3 + si + 1],
                    )
                    if state["pending"] is not None:
                        args = state["pending"]
                        state["pending"] = None
                        emit_colsum(*args)
                    if pl == 1:
                        state["pending"] = (rt // 2, et, off, w, False)

            # final pair's last colsum group trails the last exp
            p, et_pair, off, w, _ = state["pending"]
            state["pending"] = None
            emit_colsum(p, et_pair, off, w, True)

            # rt7's rowsums column: reduced separately so the first 7 could
            # be reduced early; ship on the idle scalar queue
            nc.vector.tensor_reduce(
                rsum8[:, 7:8],
                rowsums[:, 21:24].rearrange("p (a b) -> p a b", b=3),
                axis=mybir.AxisListType.X,
                op=mybir.AluOpType.add,
            )
            nc.scalar.dma_start(out=rows_d.ap()[:, :], in_=rsum8[:, :])

    _patch_act_table_loads(nc)
    nc.compile()
    return nc


def _act_set_id_with_exp(nc) -> int:
    from concourse.hw_specs import get_activation_tables

    tabs = get_activation_tables(nc.m.arch)
    for i, (name, fns) in enumerate(tabs.items()):
        if AF.Exp in fns:
            return i
    raise RuntimeError("no activation table set with Exp")


def _patch_act_table_loads(nc) -> None:
    # Load the exp table once up front instead of per-switch reloads.
    set_id = _act_set_id_with_exp(nc)

    def _single_act_table_load():
        for blk in nc.main_func.blocks:
            insts = list(blk.instructions)
            for i, ins in enumerate(insts):
                if isinstance(ins, mybir.InstActivation):
                    load = mybir.InstLoadActFuncSet(
                        name=nc.get_next_instruction_name(),
                        act_func_set_id=set_id,
                        ins=[],
                        outs=[],
                    )
                    load.engine = mybir.EngineType.Activation
                    insts.insert(i, load)
                    blk.instructions = insts
                    break

    nc.insert_act_table_loads = _single_act_table_load


def _get_nc() -> bass.Bass:
    if "nc" not in _CACHE:
        _CACHE["nc"] = _build_nc()
    return _CACHE["nc"]


def _masks():
    # lhsT constants for the mask matmuls (rhs = identity):
    # pq[p, t] += mlo[t, p], so mlo[k, m] = NEG iff k <= m masks t <= p
    # (keeps d >= 1); mup[k, m] = NEG iff k > m masks t > p (keeps d <= 4096)
    k = np.arange(128)[:, None]
    m = np.arange(128)[None, :]
    mlo = np.where(k <= m, NEG_MASK, 0.0)
    mup = np.where(k > m, NEG_MASK, 0.0)
    ident = np.eye(128)
    return np.concatenate([mlo, mup, ident], axis=1).astype(NP_F8)


def kernel(emb_i: np.ndarray, emb_j: np.ndarray) -> np.ndarray:
    global LAST_RESULTS
    z = np.concatenate(
        [np.asarray(emb_i, dtype=np.float32), np.asarray(emb_j, dtype=np.float32)],
        axis=0,
    )  # [8192, 256]
    z /= np.maximum(np.sqrt((z * z).sum(axis=1, keepdims=True)), 1e-12)
    z8 = z.astype(NP_F8)           # device values, exact
    z8f = z8.astype(np.float32)
    zt8 = np.ascontiguousarray(z8.T)  # [256, 8192] fp8
    mconst = _masks()

    in_maps = []
    for c in range(N_CORES):
        ztc = zt8 if c == 0 else np.roll(zt8, -c * ROWS, axis=1)
        ztc = ztc[:, :COLS]
        # DoubleRow plane layout: [128, 2*COLS], partition p = dims (p, p+128)
        buf = np.ascontiguousarray(
            np.concatenate([ztc[:128, :], ztc[128:, :]], axis=1)
        )
        in_maps.append({"z8": buf, "mconst": mconst})

    nc = _get_nc()
    LAST_RESULTS = run_bass_kernel_spmd(nc, in_maps, list(range(N_CORES)))

    den = np.zeros(TWO_N, dtype=np.float64)
    band_j = np.arange(BAND)
    for c in range(N_CORES):
        r = LAST_RESULTS.results[c]
        rows = np.asarray(r["rows"], dtype=np.float64)  # [128, 8] (p, rt)
        cols = np.asarray(r["cols"], dtype=np.float64)  # [8, BAND]
        den[c * ROWS : (c + 1) * ROWS] += rows.T.reshape(-1)
        for rt in range(8):
            g = (c * ROWS + rt * 128 + band_j) % TWO_N
            den[g] += cols[rt]

    idx = np.arange(TWO_N)
    pidx = (idx + N) % TWO_N
    # distance-4096 pairs were computed by both endpoints: subtract once,
    # using the same fp8 z the device saw
    pos8 = (z8f[idx] * z8f[pidx]).sum(axis=1, dtype=np.float64)
    den -= np.exp(2.0 * pos8)
    # the loss's positive term uses full-precision z
    pos = (z[idx].astype(np.float64) * z[pidx].astype(np.float64)).sum(axis=1)
    loss = np.mean(np.log(den) - 2.0 * pos)
    return np.array(loss, dtype=np.float32)


# revision 35
# speedup vs baseline: 1.7854x; 1.0050x over previous
"""NT-Xent (SimCLR) contrastive loss on 8 Trainium2 NeuronCores — v3 (fp8).

Symmetric half-band design as v2: exp(sim) is symmetric, so each global row i
only computes columns at circular distance d = j-i mod 2N in [1, 4096]; every
unordered pair lands on exactly one core except d == 4096 (the positive pair),
which lands on both and is corrected on the host.

v3 changes vs v2:
- z is normalized on the HOST (f32) and shipped as fp8e4m3 in a DoubleRow
  plane layout [128, 2, COLS] (partition p holds dims p and p+128). This
  deletes the whole on-device normalization pipeline (squares, norm matmuls,
  rsqrt chain, broadcast DMAs) that caused multi-us dependency bubbles, and
  cuts input DMA 4x.
- All matmuls run fp8 DoubleRow (0.5 cyc/row, K=256 in one pass): the sim
  matmul needs one instruction per 512 output cols, and the exp'd band tiles
  of two adjacent row-tiles are column-summed in one paired matmul.
- exp runs on ACT from [128,1536] PSUM tiles (3 per row-tile: 1536+1536+1152)
  with accum_out giving f32 row sums for free; e is written back as fp8 only
  for the colsum matmul. ACT is the bottleneck engine (~39 us busy/core).
- Column sums go per-pair straight from PSUM to DRAM (rows 2p,2p+1 of the
  [8, BAND] output), no DVE evacuation.

Host assembles den from f32 row sums + fp8-rounded col sums, subtracts the
double-counted positive exp, and takes mean(log(D) - 2*pos) with pos in f32.
"""

import sys

for _p in ("/opt/trn_rl_repo",):
    if _p not in sys.path:
        sys.path.insert(0, _p)

import ml_dtypes
import numpy as np

import concourse.bass as bass
import concourse.tile as tile
from concourse import bacc, mybir
from concourse.bass_utils import run_bass_kernel_spmd

F32 = mybir.dt.float32
F8 = mybir.dt.float8e4
AF = mybir.ActivationFunctionType
DR = mybir.MatmulPerfMode.DoubleRow
NP_F8 = ml_dtypes.float8_e4m3

N_CORES = 8
N = 4096
D = 256
TWO_N = 2 * N            # 8192 rows of sim
ROWS = TWO_N // N_CORES  # 1024 rows per core
COLS = 5120              # rotated columns staged per core
BAND = 4224              # band columns per 128-row tile (4096 + 128 wedge)
NEG_MASK = -128.0        # fp8-exact; exp(2*(sim-128)) underflows to 0
SEGS = ((0, 1536), (1536, 1536), (3072, 1152))  # band segments per row-tile

_CACHE = {}
LAST_RESULTS = None


def _plane3(base: bass.AP, off: int, plane_stride: int, w: int) -> bass.AP:
    """[128, 2, w] DoubleRow view of a plane-major [128, 2*S] sbuf tile."""
    return bass.AP(
        tensor=base.tensor,
        offset=base.offset + off,
        ap=[list(base.ap[0]), [plane_stride, 2], [1, w]],
    )


def _build_nc() -> bass.Bass:
    nc = bacc.Bacc("TRN2", num_devices=N_CORES)

    z_d = nc.dram_tensor("z8", [128, 2 * COLS], F8, kind="ExternalInput")
    # mask-matmul constants packed in one tensor: [mlo | mup | ident]
    mc_d = nc.dram_tensor("mconst", [128, 384], F8, kind="ExternalInput")
    rows_d = nc.dram_tensor("rows", [128, 8], F32, kind="ExternalOutput")
    cols_d = nc.dram_tensor("cols", [8, BAND], F32, kind="ExternalOutput")

    with tile.TileContext(nc) as tc:
        with (
            tc.tile_pool(name="big", bufs=1) as big,
            tc.tile_pool(name="ep", bufs=2) as ep,
            tc.tile_pool(name="cg", bufs=2) as cg,
            tc.tile_pool(name="small", bufs=1) as small,
            tc.tile_pool(name="ps", bufs=2, space="PSUM") as ps,
            tc.tile_pool(name="cs", bufs=2, space="PSUM") as cs,
        ):
            z8 = big.tile([128, 2 * COLS], F8, tag="z8")
            # Input DMAs: first halves of BOTH planes first (rt0 needs both),
            # spread across engine queues so the ~600ns issue costs overlap
            H = COLS // 2
            mconst = small.tile([128, 384], F8, tag="mconst")
            mask_lo = mconst[:, 0:128]
            mask_up = mconst[:, 128:256]
            ident = mconst[:, 256:384]
            # 3 chunks per plane: [0:1664] covers rt0's first segment, so
            # compute starts after ~1/3 of the load; later chunks land before
            # the row-tiles that need them
            for a, b in ((0, 1664), (1664, 3392), (3392, COLS)):
                nc.sync.dma_start(out=z8[:, a:b], in_=z_d.ap()[:, a:b])
                nc.scalar.dma_start(
                    out=z8[:, COLS + a : COLS + b],
                    in_=z_d.ap()[:, COLS + a : COLS + b],
                )
                if a == 0:
                    nc.gpsimd.dma_start(out=mconst[:, :], in_=mc_d.ap()[:, :])

            # pair selector [128, 2, 128]: plane 0 -> row 0 (even row-tile),
            # plane 1 -> row 1 (odd row-tile); every pair's colsums land at
            # psum partitions 0:2 (engine partition-base must be 0/32/64/96).
            # Full 128-col stationary: narrower ones fail the LDW ISA check.
            self_f = small.tile([128, 256], F32, tag="selftmp")
            nc.vector.memset(self_f[:, :], 0.0)
            nc.vector.memset(self_f[:, 0:1], 1.0)
            nc.vector.memset(self_f[:, 129:130], 1.0)
            sel = small.tile([128, 256], F8, tag="sel")
            nc.vector.tensor_copy(sel[:, :], self_f[:, :])

            rowsums = small.tile([128, 24], F32, tag="rsum")
            zbase = z8[:, :]

            rsum8 = small.tile([128, 8], F32, tag="rs8")

            # colsum emission is deferred by one segment: placed directly
            # after the NEXT segment's matmuls in the PE stream, the colsum
            # (which waits on its segment's exps) no longer head-of-line
            # blocks the pq fill the ACT pipeline needs next.
            state = {"cstage": None, "pending": None}

            def emit_colsum(p, et_pair, off, w, last):
                if off == 0:
                    cstage_t = cg.tile([2, BAND], F32, tag="cstage")
                    state["cstage"] = cstage_t
                cstage = state["cstage"]
                eb = et_pair[:, :]
                for ci, sub0 in enumerate(range(off, off + w, 512)):
                    sw = min(512, off + w - sub0)
                    cst = cs.tile([128, 512], F32, tag="cs")
                    nc.tensor.matmul(
                        cst[0:128, 0:sw],
                        _plane3(sel[:, :], 0, 128, 128),
                        _plane3(eb, sub0, BAND, sw),
                        start=True,
                        stop=True,
                        perf_mode=DR,
                    )
                    if last and ci % 2 == 1:
                        # ACT is done with exps by now; split the tail copies
                        # across ACT and DVE
                        nc.scalar.copy(
                            cstage[0:2, sub0 : sub0 + sw], cst[0:2, 0:sw]
                        )
                    else:
                        nc.vector.tensor_copy(
                            cstage[0:2, sub0 : sub0 + sw], cst[0:2, 0:sw]
                        )
                nc.sync.dma_start(
                    out=cols_d.ap()[2 * p : 2 * p + 2, off : off + w],
                    in_=cstage[0:2, off : off + w],
                )

            et = None
            for rt in range(8):
                if rt == 7:
                    # rowsums only lack rt7's segments; queue the reduce and
                    # its DMA (scalar queue) now so they overlap the final
                    # colsum chain instead of trailing it
                    nc.vector.tensor_reduce(
                        rsum8[:, 0:7],
                        rowsums[:, 0:21].rearrange("p (a b) -> p a b", b=3),
                        axis=mybir.AxisListType.X,
                        op=mybir.AluOpType.add,
                    )
                if rt % 2 == 0:
                    et = ep.tile([128, 2 * BAND], F8, tag="e")
                pl = rt % 2
                ebase = et[:, :]
                for si, (off, w) in enumerate(SEGS):
                    pq = ps.tile([128, 1536], F32, tag="pq")
                    lhsT = _plane3(zbase, rt * 128, COLS, 128)
                    for sub0 in range(0, w, 512):
                        sw = min(512, w - sub0)
                        # the masked sub-tiles keep their accumulation group
                        # open for the mask matmul below
                        masked = (si == 0 and sub0 == 0) or (
                            si == 2 and sub0 == 1024
                        )
                        nc.tensor.matmul(
                            pq[:, sub0 : sub0 + sw],
                            lhsT,
                            _plane3(zbase, rt * 128 + off + sub0, COLS, sw),
                            start=True,
                            stop=not masked,
                            perf_mode=DR,
                        )
                    if si == 0:
                        # first band tile: mask distance <= 0 (diag+lower):
                        # pq[p,t] += NEG*1[t<=p] as a matmul (lhsT[k,m] =
                        # NEG*1[k<=m], rhs=I), keeping the mask on the PE so
                        # no other engine gates the exp
                        nc.tensor.matmul(
                            pq[:, 0:128],
                            mask_lo,
                            ident,
                            start=False,
                            stop=True,
                            skip_group_check=True,
                        )
                    if si == 2:
                        # wedge (distance ~4096): mask d > 4096
                        nc.tensor.matmul(
                            pq[:, 1024:1152],
                            mask_up,
                            ident,
                            start=False,
                            stop=True,
                            skip_group_check=True,
                        )
                    nc.scalar.activation(
                        et[:, pl * BAND + off : pl * BAND + off + w],
                        pq[:, 0:w],
                        AF.Exp,
                        scale=2.0,
                        accum_out=rowsums[:, rt * 3 + si : rt * 3 + si + 1],
                    )
                    if state["pending"] is not None:
                        args = state["pending"]
                        state["pending"] = None
                        emit_colsum(*args)
                    if pl == 1:
                        state["pending"] = (rt // 2, et, off, w, False)

            # final pair's last colsum group trails the last exp
            p, et_pair, off, w, _ = state["pending"]
            state["pending"] = None
            emit_colsum(p, et_pair, off, w, True)

            # rt7's rowsums column: reduced separately so the first 7 could
            # be reduced early; ship on the idle scalar queue
            nc.vector.tensor_reduce(
                rsum8[:, 7:8],
                rowsums[:, 21:24].rearrange("p (a b) -> p a b", b=3),
                axis=mybir.AxisListType.X,
                op=mybir.AluOpType.add,
            )
            nc.scalar.dma_start(out=rows_d.ap()[:, :], in_=rsum8[:, :])

    _patch_act_table_loads(nc)
    nc.compile()
    return nc


def _act_set_id_with_exp(nc) -> int:
    from concourse.hw_specs import get_activation_tables

    tabs = get_activation_tables(nc.m.arch)
    for i, (name, fns) in enumerate(tabs.items()):
        if AF.Exp in fns:
            return i
    raise RuntimeError("no activation table set with Exp")


def _patch_act_table_loads(nc) -> None:
    # Load the exp table once up front instead of per-switch reloads.
    set_id = _act_set_id_with_exp(nc)

    def _single_act_table_load():
        for blk in nc.main_func.blocks:
            insts = list(blk.instructions)
            for i, ins in enumerate(insts):
                if isinstance(ins, mybir.InstActivation):
                    load = mybir.InstLoadActFuncSet(
                        name=nc.get_next_instruction_name(),
                        act_func_set_id=set_id,
                        ins=[],
                        outs=[],
                    )
                    load.engine = mybir.EngineType.Activation
                    insts.insert(i, load)
                    blk.instructions = insts
                    break

    nc.insert_act_table_loads = _single_act_table_load


def _get_nc() -> bass.Bass:
    if "nc" not in _CACHE:
        _CACHE["nc"] = _build_nc()
    return _CACHE["nc"]


def _masks():
    # lhsT constants for the mask matmuls (rhs = identity):
    # pq[p, t] += mlo[t, p], so mlo[k, m] = NEG iff k <= m masks t <= p
    # (keeps d >= 1); mup[k, m] = NEG iff k > m masks t > p (keeps d <= 4096)
    k = np.arange(128)[:, None]
    m = np.arange(128)[None, :]
    mlo = np.where(k <= m, NEG_MASK, 0.0)
    mup = np.where(k > m, NEG_MASK, 0.0)
    ident = np.eye(128)
    return np.concatenate([mlo, mup, ident], axis=1).astype(NP_F8)


def kernel(emb_i: np.ndarray, emb_j: np.ndarray) -> np.ndarray:
    global LAST_RESULTS
    z = np.concatenate(
        [np.asarray(emb_i, dtype=np.float32), np.asarray(emb_j, dtype=np.float32)],
        axis=0,
    )  # [8192, 256]
    z /= np.maximum(np.sqrt((z * z).sum(axis=1, keepdims=True)), 1e-12)
    z8 = z.astype(NP_F8)           # device values, exact
    z8f = z8.astype(np.float32)
    zt8 = np.ascontiguousarray(z8.T)  # [256, 8192] fp8
    mconst = _masks()

    in_maps = []
    for c in range(N_CORES):
        ztc = zt8 if c == 0 else np.roll(zt8, -c * ROWS, axis=1)
        ztc = ztc[:, :COLS]
        # DoubleRow plane layout: [128, 2*COLS], partition p = dims (p, p+128)
        buf = np.ascontiguousarray(
            np.concatenate([ztc[:128, :], ztc[128:, :]], axis=1)
        )
        in_maps.append({"z8": buf, "mconst": mconst})

    nc = _get_nc()
    LAST_RESULTS = run_bass_kernel_spmd(nc, in_maps, list(range(N_CORES)))

    den = np.zeros(TWO_N, dtype=np.float64)
    band_j = np.arange(BAND)
    for c in range(N_CORES):
        r = LAST_RESULTS.results[c]
        rows = np.asarray(r["rows"], dtype=np.float64)  # [128, 8] (p, rt)
        cols = np.asarray(r["cols"], dtype=np.float64)  # [8, BAND]
        den[c * ROWS : (c + 1) * ROWS] += rows.T.reshape(-1)
        for rt in range(8):
            g = (c * ROWS + rt * 128 + band_j) % TWO_N
            den[g] += cols[rt]

    idx = np.arange(TWO_N)
    pidx = (idx + N) % TWO_N
    # distance-4096 pairs were computed by both endpoints: subtract once,
    # using the same fp8 z the device saw
    pos8 = (z8f[idx] * z8f[pidx]).sum(axis=1, dtype=np.float64)
    den -= np.exp(2.0 * pos8)
    # the loss's positive term uses full-precision z
    pos = (z[idx].astype(np.float64) * z[pidx].astype(np.float64)).sum(axis=1)
    loss = np.mean(np.log(den) - 2.0 * pos)
    return np.array(loss, dtype=np.float32)
